# revision 34
# baseline (speedup 1.0000x reference)
"""GCN message-passing kernel for Trainium2 (8 NeuronCores).

Strategy:
  - Nodes sharded across 8 cores, aligned to graph boundaries (G/8 graphs/core).
  - Edges partitioned by destination shard; per layer each core computes
    z = (dinv * h) @ W for its shard, the z-table is AllGathered (bf16),
    source rows are fetched with dma_gather (4 int16-indexed sub-tables),
    and a one-hot scatter-matmul accumulates messages per 128-dst block.
  - The schedule is padded to be identical on all cores (single SPMD program).
  - Graph pooling: masked static-window reduces on h^T + one AllGather.
"""

import sys

sys.path.insert(0, "/opt/trn_rl_repo")

import numpy as np
import ml_dtypes

import concourse.bass as bass
import concourse.bacc as bacc
import concourse.tile as tile
from concourse import mybir, library_config
from concourse.bass_utils import run_bass_kernel_spmd

C = 8            # cores
P = 128          # partitions / block size
HID = 128
SBW = 5          # dst blocks per superblock
GCHUNK = 2048    # max idxs per dma_gather call
NTAB = 4         # gather sub-tables (one per shard quarter)

LAST_RESULTS = None  # set by kernel(): BassKernelResults of the last run
TRACE = False        # set True (e.g. by test.py) to capture an NTFF profile
DEBUG_DUMP = False   # dump per-layer h buffers as extra outputs


def _host_prep(x, edge_index, batch, W_emb, b_emb, W_conv, b_conv, W1, b1, W2, b2):
    N = x.shape[0]
    batch = np.asarray(batch, dtype=np.int64)
    G = int(batch.max()) + 1
    assert G % C == 0, G
    L = W_conv.shape[0]

    src = np.asarray(edge_index[0], dtype=np.int64)
    dst = np.asarray(edge_index[1], dtype=np.int64)
    self_idx = np.arange(N, dtype=np.int64)
    src = np.concatenate([src, self_idx])
    dst = np.concatenate([dst, self_idx])

    deg = np.bincount(dst, minlength=N).astype(np.float64)
    dinv = (1.0 / np.sqrt(np.maximum(deg, 1e-12))).astype(np.float32)
    dinv[deg <= 0] = 0.0

    # self-loop edges (natural + the appended ones) are handled as an
    # elementwise z_local add with per-node multiplicity, not as gathers
    keep = src != dst
    mult = np.bincount(dst[~keep], minlength=N).astype(np.float32)
    src = src[keep]
    dst = dst[keep]

    # graph-aligned sharding: core c owns graphs [c*G/C, (c+1)*G/C)
    gpc = G // C
    bounds = np.searchsorted(batch, np.arange(G + 1))
    starts = bounds[np.arange(C) * gpc]
    ends = bounds[(np.arange(C) + 1) * gpc]
    shard_sizes = ends - starts
    NB = int(np.ceil(shard_sizes.max() / P))
    # round NB up to a multiple of SBW so superblocks tile quarters evenly
    NB = int(np.ceil(NB / (4 * SBW)) * 4 * SBW)
    SHARD_PAD = NB * P
    QROWS = SHARD_PAD // NTAB
    TAB_ROWS = C * QROWS
    assert TAB_ROWS <= 32767, TAB_ROWS

    core_of = np.searchsorted(ends - 1, np.arange(N), side="left")
    loc_row = np.arange(N) - starts[core_of]
    e_tab = (loc_row // QROWS)[src].astype(np.int64)
    e_trow = (core_of * QROWS + (loc_row % QROWS))[src].astype(np.int64)
    e_core = core_of[dst]
    e_dloc = dst - starts[e_core]

    NSB = int(np.ceil(NB / SBW))

    dblk = e_dloc // P
    key = ((e_core * NB + dblk) * NTAB + e_tab).astype(np.int64)
    cnts = np.bincount(key, minlength=C * NB * NTAB)
    KD = int(np.ceil(cnts.max() / P))
    SLOT_DT = KD * P
    NMM = NB * NTAB * KD
    TOT = NMM * P

    # schedule order per core: (sb, tab, d in sb, kb)
    blocks_in_sb = [min(NB - s * SBW, SBW) for s in range(NSB)]
    grank = np.zeros((NB, NTAB), dtype=np.int64)
    acc = 0
    for s in range(NSB):
        nblk = blocks_in_sb[s]
        for t in range(NTAB):
            for j in range(nblk):
                grank[s * SBW + j, t] = acc + t * nblk + j
        acc += NTAB * nblk
    NGRP = acc
    assert NGRP == NB * NTAB

    idx_cores, dstloc_cores = [], []
    for c in range(C):
        m = e_core == c
        tab_c = e_tab[m]
        trow_c = e_trow[m]
        dloc_c = e_dloc[m]
        gr = grank[dloc_c // P, tab_c]
        order = np.argsort(gr, kind="stable")
        gr_s = gr[order]
        grp_first = np.searchsorted(gr_s, np.arange(NGRP))
        grp_cnt = np.diff(np.append(grp_first, len(gr_s)))
        assert grp_cnt.max() <= SLOT_DT, (grp_cnt.max(), SLOT_DT)
        within = np.arange(len(gr_s)) - grp_first[gr_s]
        slot = gr_s * SLOT_DT + within

        idx_arr = np.zeros(TOT, dtype=np.int16)
        idx_arr[slot] = trow_c[order].astype(np.int16)
        dl_arr = np.full(TOT, -1.0, dtype=np.float32)
        dl_arr[slot] = (dloc_c[order] % P).astype(np.float32)

        idx_wrapped = np.tile(
            np.ascontiguousarray(idx_arr.reshape(-1, 16).T), (C, 1)
        )
        dstloc = np.ascontiguousarray(
            dl_arr.reshape(NMM, P).T.astype(ml_dtypes.bfloat16)
        )
        idx_cores.append(idx_wrapped)
        dstloc_cores.append(dstloc)

    # pooling windows: static lo/width per local graph, masked by gid data
    lo_u = np.zeros(gpc, dtype=np.int64)
    wd_u = np.zeros(gpc, dtype=np.int64)
    for g in range(gpc):
        los = bounds[np.arange(C) * gpc + g] - starts
        his = bounds[np.arange(C) * gpc + g + 1] - starts
        lo_u[g] = los.min()
        wd_u[g] = max(his.max() - lo_u[g], 1)
    HT_W = int(max(SHARD_PAD, (lo_u + wd_u).max()))

    gid_cores = []
    for c in range(C):
        gid = np.full(HT_W, -1.0, dtype=np.float32)
        n = shard_sizes[c]
        gid[:n] = (batch[starts[c]:ends[c]] - c * gpc).astype(np.float32)
        gid_cores.append(
            np.ascontiguousarray(np.tile(gid.astype(ml_dtypes.bfloat16), (P, 1)))
        )

    cnt = (bounds[1:] - bounds[:-1]).astype(np.float32)
    cntinv = (1.0 / np.maximum(cnt, 1.0)).astype(np.float32)

    xT_cores, dinv_cores, mult_cores = [], [], []
    D_IN = x.shape[1]
    for c in range(C):
        xs = np.zeros((SHARD_PAD, D_IN), dtype=np.float32)
        xs[: shard_sizes[c]] = x[starts[c]:ends[c]]
        xT_cores.append(np.ascontiguousarray(xs.T))
        dv = np.zeros(SHARD_PAD, dtype=np.float32)
        dv[: shard_sizes[c]] = dinv[starts[c]:ends[c]]
        dinv_cores.append(np.ascontiguousarray(dv.reshape(NB, P).T))
        mu = np.zeros(SHARD_PAD, dtype=np.float32)
        mu[: shard_sizes[c]] = mult[starts[c]:ends[c]]
        mult_cores.append(np.ascontiguousarray(mu.reshape(NB, P).T))

    cfg = dict(
        N=N, G=G, L=L, gpc=gpc, NB=NB, SHARD_PAD=SHARD_PAD, TAB_ROWS=TAB_ROWS,
        NSB=NSB, KD=KD, SLOT_DT=SLOT_DT, NMM=NMM, TOT=TOT, D_IN=D_IN,
        HT_W=HT_W, lo_u=lo_u.tolist(), wd_u=wd_u.tolist(),
        blocks_in_sb=blocks_in_sb,
        D_OUT=W2.shape[1], H1=W1.shape[1],
    )

    common = dict(
        W_emb=np.asarray(W_emb, np.float32),
        W_conv=np.asarray(W_conv, np.float32).reshape(L * HID, HID).astype(ml_dtypes.bfloat16),
        W1=np.asarray(W1, np.float32),
        W2=np.asarray(W2, np.float32),
        b_emb_b=np.tile(np.asarray(b_emb, np.float32), (P, 1)),
        b_conv_b=np.tile(
            np.asarray(b_conv, np.float32)[:, None, :], (1, P, 1)
        ).reshape(L * P, HID),
        b1_b=np.tile(np.asarray(b1, np.float32), (P, 1)),
        b2_b=np.tile(np.asarray(b2, np.float32), (P, 1)),
        iota=np.tile(np.arange(P, dtype=np.float32), (P, 1)).astype(
            ml_dtypes.bfloat16
        ),
        ident=np.eye(P, dtype=np.float32),
        identb=np.eye(P, dtype=np.float32).astype(ml_dtypes.bfloat16),
        cntinv=cntinv.reshape(G, 1),
    )
    per_core = [
        dict(
            xT=xT_cores[c], dinv_t=dinv_cores[c], idx=idx_cores[c],
            dstloc=dstloc_cores[c], gid=gid_cores[c], mult_t=mult_cores[c],
        )
        for c in range(C)
    ]
    return cfg, common, per_core


def _build(cfg):
    G, L = cfg["G"], cfg["L"]
    gpc, NB, SHARD_PAD = cfg["gpc"], cfg["NB"], cfg["SHARD_PAD"]
    TAB_ROWS, NSB, KD = cfg["TAB_ROWS"], cfg["NSB"], cfg["KD"]
    SLOT_DT, NMM, TOT = cfg["SLOT_DT"], cfg["NMM"], cfg["TOT"]
    D_IN, HT_W = cfg["D_IN"], cfg["HT_W"]
    blocks_in_sb = cfg["blocks_in_sb"]
    D_OUT, H1 = cfg["D_OUT"], cfg["H1"]
    H1H = H1 // 2
    WDMAX = int(max(cfg["wd_u"]))
    f32, bf16, i16 = mybir.dt.float32, mybir.dt.bfloat16, mybir.dt.int16
    AFT = mybir.ActivationFunctionType
    Alu = mybir.AluOpType

    nc = bacc.Bacc(
        "TRN2", target_bir_lowering=False, debug=False, num_devices=C,
        num_swdge_queues=4,
    )

    xT = nc.dram_tensor("xT", [D_IN, SHARD_PAD], f32, kind="ExternalInput")
    W_emb = nc.dram_tensor("W_emb", [D_IN, HID], f32, kind="ExternalInput")
    W_conv = nc.dram_tensor("W_conv", [L * HID, HID], bf16, kind="ExternalInput")
    W1 = nc.dram_tensor("W1", [3 * HID, H1], f32, kind="ExternalInput")
    W2 = nc.dram_tensor("W2", [H1, D_OUT], f32, kind="ExternalInput")
    b_emb_b = nc.dram_tensor("b_emb_b", [P, HID], f32, kind="ExternalInput")
    b_conv_b = nc.dram_tensor("b_conv_b", [L * P, HID], f32, kind="ExternalInput")
    b1_b = nc.dram_tensor("b1_b", [P, H1], f32, kind="ExternalInput")
    b2_b = nc.dram_tensor("b2_b", [P, D_OUT], f32, kind="ExternalInput")
    iota_d = nc.dram_tensor("iota", [P, P], bf16, kind="ExternalInput")
    ident_d = nc.dram_tensor("ident", [P, P], f32, kind="ExternalInput")
    identb_d = nc.dram_tensor("identb", [P, P], bf16, kind="ExternalInput")
    cntinv_d = nc.dram_tensor("cntinv", [G, 1], f32, kind="ExternalInput")
    dinv_d = nc.dram_tensor("dinv_t", [P, NB], f32, kind="ExternalInput")
    mult_d = nc.dram_tensor("mult_t", [P, NB], f32, kind="ExternalInput")
    idx_d = nc.dram_tensor("idx", [P, TOT // 16], i16, kind="ExternalInput")
    dstloc_d = nc.dram_tensor("dstloc", [P, NMM], bf16, kind="ExternalInput")
    gid_d = nc.dram_tensor("gid", [P, HT_W], bf16, kind="ExternalInput")
    out_d = nc.dram_tensor("out", [G, D_OUT], f32, kind="ExternalOutput")

    QROWS = SHARD_PAD // NTAB
    z_local = nc.dram_tensor("z_local", [SHARD_PAD, HID], bf16, kind="Internal")
    z_tabs = [
        [
            nc.dram_tensor(f"z_tab{t}_{i}", [C * QROWS, HID], bf16, kind="Internal")
            for i in range(2)
        ]
        for t in range(NTAB)
    ]
    pool_loc = nc.dram_tensor("pool_loc", [gpc, 2 * HID], f32, kind="Internal")
    pool_all = nc.dram_tensor("pool_all", [G, 2 * HID], f32, kind="Internal")
    dumps = (
        [
            nc.dram_tensor(f"hdump{i}", [P, SHARD_PAD], f32, kind="ExternalOutput")
            for i in range(L + 1)
        ]
        if DEBUG_DUMP
        else None
    )

    rg = [list(range(C))]

    with tile.TileContext(nc) as tc:
        with (
            tc.tile_pool(name="const", bufs=1) as cpool,
            tc.tile_pool(name="big", bufs=1) as bigpool,
            tc.tile_pool(name="g", bufs=14) as gpool,
            tc.tile_pool(name="s", bufs=2) as spool,
            tc.tile_pool(name="ix", bufs=12) as ixpool,
            tc.tile_pool(name="work", bufs=2) as wpool,
            tc.tile_pool(name="zst", bufs=2) as zpool,
            tc.tile_pool(name="zself", bufs=2) as zspool,
            tc.tile_pool(name="ps", bufs=3, space="PSUM") as pspool,
            tc.tile_pool(name="agg", bufs=5, space="PSUM") as aggpool,
        ):
            nc.gpsimd.load_library(library_config.mlp)

            def cload(dram_ap, shape, dtype, nm):
                t = cpool.tile(shape, dtype, name=nm, tag=nm)
                nc.sync.dma_start(t[:], dram_ap)
                return t

            Wemb_s = cload(W_emb[:], [D_IN, HID], f32, "Wemb_s")
            Wc_s = cload(
                W_conv[:].rearrange("(l k) h -> k l h", k=P), [P, L, HID], bf16
            , "Wc_s")
            W1_s = cload(W1[:].rearrange("(a k) h -> k a h", k=P), [P, 3, H1], f32, "W1_s")
            W2a_s = cload(W2[0:H1H, :], [H1H, D_OUT], f32, "W2a_s")
            W2b_s = cload(W2[H1H:H1, :], [H1H, D_OUT], f32, "W2b_s")
            bemb_s = cload(b_emb_b[:], [P, HID], f32, "bemb_s")
            mult_s = cload(mult_d[:], [P, NB], f32, "mult_s")
            bconv_s = cload(
                b_conv_b[:].rearrange("(l k) h -> k l h", k=P), [P, L, HID], f32
            , "bconv_s")
            b1_s = cload(b1_b[:], [P, H1], f32, "b1_s")
            b2_s = cload(b2_b[:], [P, D_OUT], f32, "b2_s")
            iota_s = cload(iota_d[:], [P, P], bf16, "iota_s")
            ident_s = cload(ident_d[:], [P, P], f32, "ident_s")
            identb_s = cload(identb_d[:], [P, P], bf16, "identb_s")
            cnt_s = cload(cntinv_d[:], [G, 1], f32, "cnt_s")
            dinv_s = cload(dinv_d[:], [P, NB], f32, "dinv_s")
            dstloc_s = cload(dstloc_d[:], [P, NMM], bf16, "dstloc_s")

            hbuf = bigpool.tile([P, SHARD_PAD], bf16, tag="h")

            # ---- helpers shared by embed and the layer loop ----
            QB = NB // NTAB            # blocks per quarter
            SBQ = QB // SBW            # superblocks per quarter

            def z_quarter(w_idx, qq):
                """z rows for quarter qq from hbuf via W_conv[w_idx]."""
                for b8 in range(qq * QB, (qq + 1) * QB, SBW):
                    zstage = zpool.tile([P, SBW * HID], bf16, tag="zst")
                    for j in range(SBW):
                        b = b8 + j
                        pst = pspool.tile([P, P], bf16, tag="ps")
                        nc.tensor.transpose(
                            out=pst[:], in_=hbuf[:, b * P:(b + 1) * P],
                            identity=identb_s[:],
                        )
                        hT_b = wpool.tile([P, P], bf16, tag="hTb")
                        nc.vector.tensor_copy(hT_b[:], pst[:])
                        psz = pspool.tile([P, HID], f32, tag="ps")
                        nc.tensor.matmul(
                            psz[:], lhsT=hT_b[:], rhs=Wc_s[:, w_idx, :],
                            start=True, stop=True,
                        )
                        nc.vector.tensor_copy(
                            zstage[:, j * HID:(j + 1) * HID], psz[:]
                        )
                    nc.sync.dma_start(
                        z_local[b8 * P:(b8 + SBW) * P, :].rearrange(
                            "(b p) h -> p b h", p=P
                        ),
                        zstage[:].rearrange("p (b h) -> p b h", h=HID),
                    )

            def ag_quarter(qq, parity):
                nc.gpsimd.collective_compute(
                    "AllGather", Alu.bypass,
                    replica_groups=rg,
                    ins=[z_local[qq * QROWS:(qq + 1) * QROWS, :].opt()],
                    outs=[z_tabs[qq][parity][:].opt()],
                )

            def epilogue_quarter(l, qq):
                """h_q = tanh(dinv*agg_q + b); if l<L-1 also *= dinv."""
                c0, c1 = qq * QB, (qq + 1) * QB
                hq = hbuf[:, c0 * P:c1 * P]
                dq = dinv_s[:, c0:c1]
                nc.vector.tensor_tensor(
                    out=hq.rearrange("p (b k) -> p b k", k=P),
                    in0=hq.rearrange("p (b k) -> p b k", k=P),
                    in1=dq.to_broadcast([P, QB, P]),
                    op=Alu.mult,
                )
                nc.vector.tensor_tensor(
                    out=hq.rearrange("p (b h) -> p b h", h=HID),
                    in0=hq.rearrange("p (b h) -> p b h", h=HID),
                    in1=bconv_s[:, l, :].rearrange(
                        "p (a h) -> p a h", a=1
                    ).to_broadcast([P, QB, HID]),
                    op=Alu.add,
                )
                nc.scalar.activation(hq, hq, AFT.Tanh)
                if l < L - 1:
                    nc.vector.tensor_tensor(
                        out=hq.rearrange("p (b k) -> p b k", k=P),
                        in0=hq.rearrange("p (b k) -> p b k", k=P),
                        in1=dq.to_broadcast([P, QB, P]),
                        op=Alu.mult,
                    )

            # ---- embed: h'0 = dinv * (x @ W_emb + b_emb), pipelined with
            # z(0) + its AllGather per completed quarter ----
            nextq = 0
            for b4 in range(0, NB, 4):
                nb4 = min(4, NB - b4)
                ps = pspool.tile([P, 4 * HID], f32, tag="ps")
                for j in range(nb4):
                    b = b4 + j
                    xt_b = wpool.tile([D_IN, P], f32, tag="xt")
                    nc.sync.dma_start(xt_b[:], xT[:, b * P:(b + 1) * P])
                    nc.tensor.matmul(
                        ps[:, j * HID:(j + 1) * HID],
                        lhsT=xt_b[:], rhs=Wemb_s[:],
                        start=True, stop=True,
                    )
                nc.vector.tensor_copy(
                    hbuf[:, b4 * P: b4 * P + nb4 * HID], ps[:, : nb4 * HID]
                )
                nc.vector.tensor_tensor(
                    out=hbuf[:, b4 * P: (b4 + nb4) * P].rearrange(
                        "p (b h) -> p b h", h=HID
                    ),
                    in0=hbuf[:, b4 * P: (b4 + nb4) * P].rearrange(
                        "p (b h) -> p b h", h=HID
                    ),
                    in1=bemb_s[:].rearrange(
                        "p (a h) -> p a h", a=1
                    ).to_broadcast([P, nb4, HID]),
                    op=Alu.add,
                )
                nc.vector.tensor_tensor(
                    out=hbuf[:, b4 * P: (b4 + nb4) * P].rearrange(
                        "p (b k) -> p b k", k=P
                    ),
                    in0=hbuf[:, b4 * P: (b4 + nb4) * P].rearrange(
                        "p (b k) -> p b k", k=P
                    ),
                    in1=dinv_s[:, b4: b4 + nb4].to_broadcast([P, nb4, P]),
                    op=Alu.mult,
                )
                while nextq < NTAB and (nextq + 1) * QB <= b4 + nb4:
                    z_quarter(0, nextq)
                    ag_quarter(nextq, 0)
                    nextq += 1
            if dumps is not None:
                nc.sync.dma_start(dumps[0][:], hbuf[:])

            # ---- layers ----
            # AG(l, 3) is dispatched inside layer l's first superblock just
            # before its tab-3 gathers, hiding its latency under tabs 0-2
            ag3_pending = None
            for l in range(L):
                # AllGathers for layer l+1, deferred a few sbs so the pool
                # stream isn't head-of-line blocked on z being ready
                agq = []

                mcol = 0
                slot0 = 0
                for sb in range(NSB):
                    while agq and agq[0][0] <= sb:
                        _, qq = agq.pop(0)
                        ag_quarter(qq, (l + 1) % 2)
                    nblk = blocks_in_sb[sb]
                    aggs = [
                        aggpool.tile([P, HID], f32, tag="agg",
                                     name=f"agg_{l}_{sb}_{i}")
                        for i in range(nblk)
                    ]
                    sb_mcol = mcol
                    for ti, t in enumerate(range(NTAB)):
                        if t == 3 and ag3_pending is not None:
                            ag_quarter(3, ag3_pending)
                            ag3_pending = None
                        seg = nblk * SLOT_DT
                        t_slot0 = slot0 + t * seg
                        idxt = ixpool.tile(
                            [P, SBW * SLOT_DT // 16], i16, tag="ix"
                        )
                        nc.sync.dma_start(
                            idxt[:, : seg // 16],
                            idx_d[:, t_slot0 // 16:(t_slot0 + seg) // 16],
                        )
                        GC = GCHUNK
                        gtiles = []
                        off = 0
                        while off < seg:
                            n = min(GC, seg - off)
                            g = gpool.tile([P, GC // P, HID], bf16, tag="g")
                            nc.gpsimd.dma_gather(
                                g[:, : n // P, :],
                                z_tabs[t][l % 2][:],
                                idxt[:, off // 16:(off + n) // 16],
                                n, n, HID, single_packet=False,
                                queue_num=t,
                            )
                            gtiles.append(g)
                            off += n
                        m0 = sb_mcol + t * nblk * KD
                        sbt = spool.tile([P, SBW * KD * P], bf16, tag="s")
                        nc.vector.tensor_tensor(
                            out=sbt[:, : nblk * KD * P],
                            in0=dstloc_s[:, m0:m0 + nblk * KD].to_broadcast(
                                [P, nblk * KD, P]
                            ),
                            in1=iota_s[:].rearrange(
                                "p (a k) -> p a k", a=1
                            ).to_broadcast([P, nblk * KD, P]),
                            op=Alu.is_equal,
                        )
                        for di in range(nblk):
                            for kb in range(KD):
                                srel = (di * KD + kb) * P
                                ci, col = srel // GC, (srel % GC) // P
                                kk = di * KD + kb
                                nc.tensor.matmul(
                                    aggs[di][:],
                                    lhsT=sbt[:, kk * P:(kk + 1) * P],
                                    rhs=gtiles[ci][:, col, :],
                                    start=(ti == 0 and kb == 0),
                                    stop=(ti == NTAB - 1 and kb == KD - 1),
                                )
                    slot0 += NTAB * nblk * SLOT_DT
                    mcol += NTAB * nblk * KD
                    # self-loop contribution: hbuf_blk = agg + mult * z_local_blk
                    zs = zspool.tile([P, SBW * HID], bf16, tag="zs")
                    nc.sync.dma_start(
                        zs[:, : nblk * HID].rearrange("p (b h) -> p b h", h=HID),
                        z_local[
                            sb * SBW * P:(sb * SBW + nblk) * P, :
                        ].rearrange("(b p) h -> p b h", p=P),
                    )
                    zmul = zspool.tile([P, SBW * HID], bf16, tag="zmul")
                    for ai, a in enumerate(aggs):
                        b = sb * SBW + ai
                        nc.vector.tensor_scalar(
                            zmul[:, ai * HID:(ai + 1) * HID],
                            zs[:, ai * HID:(ai + 1) * HID],
                            mult_s[:, b:b + 1], None, Alu.mult,
                        )
                        nc.vector.tensor_tensor(
                            out=hbuf[:, b * P:(b + 1) * P],
                            in0=a[:],
                            in1=zmul[:, ai * HID:(ai + 1) * HID],
                            op=Alu.add,
                        )
                    # quarter finished → epilogue + next layer's z + deferred AG
                    if (sb + 1) % SBQ == 0:
                        qq = (sb + 1) // SBQ - 1
                        epilogue_quarter(l, qq)
                        if l < L - 1:
                            z_quarter(l + 1, qq)
                            if qq < NTAB - 1:
                                agq.append((sb + 3, qq))
                for _, qq in agq:
                    ag_quarter(qq, (l + 1) % 2)
                if l < L - 1:
                    ag3_pending = (l + 1) % 2
                if dumps is not None:
                    nc.sync.dma_start(dumps[l + 1][:], hbuf[:])

            # ---- pooling ----
            hT = bigpool.tile([P, HT_W], bf16, tag="hT")
            if HT_W > SHARD_PAD:
                nc.vector.memset(hT[:, SHARD_PAD:], 0.0)
            for b in range(NB):
                pst = pspool.tile([P, P], bf16, tag="ps")
                nc.tensor.transpose(
                    out=pst[:], in_=hbuf[:, b * P:(b + 1) * P],
                    identity=identb_s[:],
                )
                nc.vector.tensor_copy(hT[:, b * P:(b + 1) * P], pst[:])
            gid_s = bigpool.tile([P, HT_W], bf16, tag="gid")
            nc.sync.dma_start(gid_s[:], gid_d[:])

            sumP = wpool.tile([P, gpc], f32, tag="sumP")
            maxP = wpool.tile([P, gpc], f32, tag="maxP")
            for g in range(gpc):
                lo, wd = cfg["lo_u"][g], cfg["wd_u"][g]
                eq = wpool.tile([P, WDMAX], bf16, tag="eq")
                nc.vector.tensor_scalar(
                    eq[:, :wd], gid_s[:, lo:lo + wd], float(g), None,
                    Alu.is_equal,
                )
                msk = wpool.tile([P, WDMAX], f32, tag="msk")
                nc.vector.tensor_tensor(
                    out=msk[:, :wd], in0=hT[:, lo:lo + wd], in1=eq[:, :wd],
                    op=Alu.mult,
                )
                nc.vector.reduce_sum(
                    sumP[:, g:g + 1], msk[:, :wd], axis=mybir.AxisListType.X
                )
                nc.vector.tensor_scalar(
                    msk[:, :wd], eq[:, :wd], 60.0, -60.0, Alu.mult, Alu.add
                )
                nc.vector.tensor_tensor(
                    out=msk[:, :wd], in0=hT[:, lo:lo + wd], in1=msk[:, :wd],
                    op=Alu.add,
                )
                nc.vector.reduce_max(
                    maxP[:, g:g + 1], msk[:, :wd], axis=mybir.AxisListType.X
                )
            pg = pspool.tile([P, 2 * HID], f32, tag="ps")
            nc.tensor.transpose(
                out=pg[:gpc, :HID], in_=sumP[:], identity=ident_s[:]
            )
            nc.tensor.transpose(
                out=pg[:gpc, HID:], in_=maxP[:], identity=ident_s[:]
            )
            pl = wpool.tile([gpc, 2 * HID], f32, tag="pl")
            nc.vector.tensor_copy(pl[:], pg[:gpc, :])
            nc.sync.dma_start(pool_loc[:], pl[:])
            nc.gpsimd.collective_compute(
                "AllGather", Alu.bypass, replica_groups=rg,
                ins=[pool_loc[:].opt()], outs=[pool_all[:].opt()],
            )
            pa = wpool.tile([G, 2 * HID], f32, tag="pa")
            nc.sync.dma_start(pa[:], pool_all[:])
            mfix = wpool.tile([G, HID], f32, tag="mfix")
            nc.vector.tensor_scalar(
                mfix[:], pa[:, HID:], -50.0, None, Alu.is_gt
            )
            nc.vector.tensor_tensor(
                out=pa[:, HID:], in0=pa[:, HID:], in1=mfix[:], op=Alu.mult
            )
            mean_gf = wpool.tile([G, HID], f32, tag="mean")
            nc.vector.tensor_scalar(
                mean_gf[:], pa[:, :HID], cnt_s[:, :1], None, Alu.mult
            )
            gT = wpool.tile([P, 3 * G], f32, tag="gT")
            for a, src_ap in enumerate([pa[:, :HID], pa[:, HID:], mean_gf[:]]):
                ptx = pspool.tile([P, G], f32, tag="ps")
                nc.tensor.transpose(
                    out=ptx[:, :G], in_=src_ap, identity=ident_s[:G, :G]
                )
                nc.vector.tensor_copy(gT[:, a * G:(a + 1) * G], ptx[:, :G])

            # ---- head ----
            ph1 = pspool.tile([G, H1], f32, tag="ps")
            for a in range(3):
                nc.tensor.matmul(
                    ph1[:], lhsT=gT[:, a * G:(a + 1) * G], rhs=W1_s[:, a, :],
                    start=(a == 0), stop=(a == 2),
                )
            g1 = wpool.tile([G, H1], f32, tag="g1")
            nc.vector.tensor_tensor(
                out=g1[:], in0=ph1[:], in1=b1_s[:G, :], op=Alu.add
            )
            nc.scalar.activation(g1[:], g1[:], AFT.Lrelu, alpha=0.01)
            g1T = wpool.tile([H1H, 2 * G], f32, tag="g1T")
            for a in range(2):
                ptt = pspool.tile([H1H, G], f32, tag="ps")
                nc.tensor.transpose(
                    out=ptt[:], in_=g1[:, a * H1H:(a + 1) * H1H],
                    identity=ident_s[:G, :G],
                )
                nc.vector.tensor_copy(g1T[:, a * G:(a + 1) * G], ptt[:])
            ph2 = pspool.tile([G, D_OUT], f32, tag="ps")
            for a in range(2):
                nc.tensor.matmul(
                    ph2[:], lhsT=g1T[:, a * G:(a + 1) * G],
                    rhs=(W2a_s if a == 0 else W2b_s)[:],
                    start=(a == 0), stop=(a == 1),
                )
            go = wpool.tile([G, D_OUT], f32, tag="go")
            nc.vector.tensor_tensor(
                out=go[:], in0=ph2[:], in1=b2_s[:G, :], op=Alu.add
            )
            nc.scalar.activation(go[:], go[:], AFT.Lrelu, alpha=0.01)
            nc.sync.dma_start(out_d[:], go[:])

    nc.compile()
    return nc


def _install_ntff_shim():
    """Provide antenv.axon_hooks (missing in this image) so that
    run_bass_kernel_spmd(trace=True) can capture an NTFF profile via the
    injected libaxon_pjrt.so. Only used when TRACE=True."""
    import types
    import ctypes
    import contextlib

    try:
        from antenv.axon_hooks import get_axon_ntff_profile_hook  # noqa: F401
        return
    except ImportError:
        pass
    so_path = "/opt/axon/libaxon_pjrt.so"
    try:
        lib = ctypes.CDLL(so_path)
    except OSError:
        return
    if not hasattr(lib, "axon_start_nrt_profile"):
        return
    lib.axon_start_nrt_profile.argtypes = [
        ctypes.POINTER(ctypes.c_int64), ctypes.c_size_t,
    ]
    lib.axon_start_nrt_profile.restype = ctypes.c_int64
    lib.axon_stop_nrt_profile.argtypes = [ctypes.c_char_p]
    lib.axon_stop_nrt_profile.restype = ctypes.c_int64

    @contextlib.contextmanager
    def _hook(output_dir, device_ids):
        import jax
        jax.devices()
        if device_ids:
            ids = (ctypes.c_int64 * len(device_ids))(*device_ids)
            rc = lib.axon_start_nrt_profile(ids, len(device_ids))
        else:
            rc = lib.axon_start_nrt_profile(None, 0)
        if rc != 0:
            raise RuntimeError(f"axon_start_nrt_profile rc={rc}")
        try:
            yield
        finally:
            n = lib.axon_stop_nrt_profile(str(output_dir).encode())
            print(f"profile: {n} file(s) written to {output_dir}",
                  file=sys.stderr)

    mod = types.ModuleType("antenv.axon_hooks")
    mod.get_axon_ntff_profile_hook = lambda: _hook
    mod.set_axon_ntff_profile_hook = lambda h: None
    sys.modules["antenv.axon_hooks"] = mod


def kernel(**inputs):
    global LAST_RESULTS
    if TRACE:
        _install_ntff_shim()
    ins = {k: np.asarray(v) for k, v in inputs.items()}
    cfg, common, per_core = _host_prep(
        ins["x"].astype(np.float32), ins["edge_index"], ins["batch"],
        ins["W_emb"], ins["b_emb"], ins["W_conv"], ins["b_conv"],
        ins["W1"], ins["b1"], ins["W2"], ins["b2"],
    )
    nc = _build(cfg)

    in_maps = []
    for c in range(C):
        m = dict(
            xT=per_core[c]["xT"],
            W_emb=common["W_emb"], W_conv=common["W_conv"],
            W1=common["W1"], W2=common["W2"],
            b_emb_b=common["b_emb_b"], b_conv_b=common["b_conv_b"],
            b1_b=common["b1_b"], b2_b=common["b2_b"],
            iota=common["iota"], ident=common["ident"],
            identb=common["identb"],
            cntinv=common["cntinv"],
            dinv_t=per_core[c]["dinv_t"], idx=per_core[c]["idx"],
            dstloc=per_core[c]["dstloc"], gid=per_core[c]["gid"],
            mult_t=per_core[c]["mult_t"],
        )
        in_maps.append(m)

    res = run_bass_kernel_spmd(
        nc, in_maps, core_ids=list(range(C)), trace=TRACE
    )
    LAST_RESULTS = res
    return np.asarray(res.results[0]["out"], dtype=np.float32)



# revision 35
# speedup vs baseline: 1.0099x; 1.0099x over previous
"""GCN message-passing kernel for Trainium2 (8 NeuronCores).

Strategy:
  - Nodes sharded across 8 cores, aligned to graph boundaries (G/8 graphs/core).
  - Edges partitioned by destination shard; per layer each core computes
    z = (dinv * h) @ W for its shard, the z-table is AllGathered (bf16),
    source rows are fetched with dma_gather (4 int16-indexed sub-tables),
    and a one-hot scatter-matmul accumulates messages per 128-dst block.
  - The schedule is padded to be identical on all cores (single SPMD program).
  - Graph pooling: masked static-window reduces on h^T + one AllGather.
"""

import sys

sys.path.insert(0, "/opt/trn_rl_repo")

import numpy as np
import ml_dtypes

import concourse.bass as bass
import concourse.bacc as bacc
import concourse.tile as tile
from concourse import mybir, library_config
from concourse.bass_utils import run_bass_kernel_spmd

C = 8            # cores
P = 128          # partitions / block size
HID = 128
SBW = 5          # dst blocks per superblock
GCHUNK = 2048    # max idxs per dma_gather call
NTAB = 4         # gather sub-tables (one per shard quarter)

LAST_RESULTS = None  # set by kernel(): BassKernelResults of the last run
TRACE = False        # set True (e.g. by test.py) to capture an NTFF profile
DEBUG_DUMP = False   # dump per-layer h buffers as extra outputs


def _host_prep(x, edge_index, batch, W_emb, b_emb, W_conv, b_conv, W1, b1, W2, b2):
    N = x.shape[0]
    batch = np.asarray(batch, dtype=np.int64)
    G = int(batch.max()) + 1
    assert G % C == 0, G
    L = W_conv.shape[0]

    src = np.asarray(edge_index[0], dtype=np.int64)
    dst = np.asarray(edge_index[1], dtype=np.int64)
    self_idx = np.arange(N, dtype=np.int64)
    src = np.concatenate([src, self_idx])
    dst = np.concatenate([dst, self_idx])

    deg = np.bincount(dst, minlength=N).astype(np.float64)
    dinv = (1.0 / np.sqrt(np.maximum(deg, 1e-12))).astype(np.float32)
    dinv[deg <= 0] = 0.0

    # self-loop edges (natural + the appended ones) are handled as an
    # elementwise z_local add with per-node multiplicity, not as gathers
    keep = src != dst
    mult = np.bincount(dst[~keep], minlength=N).astype(np.float32)
    src = src[keep]
    dst = dst[keep]

    # graph-aligned sharding: core c owns graphs [c*G/C, (c+1)*G/C)
    gpc = G // C
    bounds = np.searchsorted(batch, np.arange(G + 1))
    starts = bounds[np.arange(C) * gpc]
    ends = bounds[(np.arange(C) + 1) * gpc]
    shard_sizes = ends - starts
    NB = int(np.ceil(shard_sizes.max() / P))
    # round NB up to a multiple of SBW so superblocks tile quarters evenly
    NB = int(np.ceil(NB / (4 * SBW)) * 4 * SBW)
    SHARD_PAD = NB * P
    QROWS = SHARD_PAD // NTAB
    TAB_ROWS = C * QROWS
    assert TAB_ROWS <= 32767, TAB_ROWS

    core_of = np.searchsorted(ends - 1, np.arange(N), side="left")
    loc_row = np.arange(N) - starts[core_of]
    e_tab = (loc_row // QROWS)[src].astype(np.int64)
    e_trow = (core_of * QROWS + (loc_row % QROWS))[src].astype(np.int64)
    e_core = core_of[dst]
    e_dloc = dst - starts[e_core]

    NSB = int(np.ceil(NB / SBW))

    dblk = e_dloc // P
    key = ((e_core * NB + dblk) * NTAB + e_tab).astype(np.int64)
    cnts = np.bincount(key, minlength=C * NB * NTAB)
    KD = int(np.ceil(cnts.max() / P))
    SLOT_DT = KD * P
    NMM = NB * NTAB * KD
    TOT = NMM * P

    # schedule order per core: (sb, tab, d in sb, kb)
    blocks_in_sb = [min(NB - s * SBW, SBW) for s in range(NSB)]
    grank = np.zeros((NB, NTAB), dtype=np.int64)
    acc = 0
    for s in range(NSB):
        nblk = blocks_in_sb[s]
        for t in range(NTAB):
            for j in range(nblk):
                grank[s * SBW + j, t] = acc + t * nblk + j
        acc += NTAB * nblk
    NGRP = acc
    assert NGRP == NB * NTAB

    idx_cores, dstloc_cores = [], []
    for c in range(C):
        m = e_core == c
        tab_c = e_tab[m]
        trow_c = e_trow[m]
        dloc_c = e_dloc[m]
        gr = grank[dloc_c // P, tab_c]
        order = np.argsort(gr, kind="stable")
        gr_s = gr[order]
        grp_first = np.searchsorted(gr_s, np.arange(NGRP))
        grp_cnt = np.diff(np.append(grp_first, len(gr_s)))
        assert grp_cnt.max() <= SLOT_DT, (grp_cnt.max(), SLOT_DT)
        within = np.arange(len(gr_s)) - grp_first[gr_s]
        slot = gr_s * SLOT_DT + within

        idx_arr = np.zeros(TOT, dtype=np.int16)
        idx_arr[slot] = trow_c[order].astype(np.int16)
        dl_arr = np.full(TOT, -1.0, dtype=np.float32)
        dl_arr[slot] = (dloc_c[order] % P).astype(np.float32)

        idx_wrapped = np.tile(
            np.ascontiguousarray(idx_arr.reshape(-1, 16).T), (C, 1)
        )
        dstloc = np.ascontiguousarray(
            dl_arr.reshape(NMM, P).T.astype(ml_dtypes.bfloat16)
        )
        idx_cores.append(idx_wrapped)
        dstloc_cores.append(dstloc)

    # pooling windows: static lo/width per local graph, masked by gid data
    lo_u = np.zeros(gpc, dtype=np.int64)
    wd_u = np.zeros(gpc, dtype=np.int64)
    for g in range(gpc):
        los = bounds[np.arange(C) * gpc + g] - starts
        his = bounds[np.arange(C) * gpc + g + 1] - starts
        lo_u[g] = los.min()
        wd_u[g] = max(his.max() - lo_u[g], 1)
    HT_W = int(max(SHARD_PAD, (lo_u + wd_u).max()))

    gid_cores = []
    for c in range(C):
        gid = np.full(HT_W, -1.0, dtype=np.float32)
        n = shard_sizes[c]
        gid[:n] = (batch[starts[c]:ends[c]] - c * gpc).astype(np.float32)
        gid_cores.append(
            np.ascontiguousarray(np.tile(gid.astype(ml_dtypes.bfloat16), (P, 1)))
        )

    cnt = (bounds[1:] - bounds[:-1]).astype(np.float32)
    cntinv = (1.0 / np.maximum(cnt, 1.0)).astype(np.float32)

    xT_cores, dinv_cores, mult_cores = [], [], []
    D_IN = x.shape[1]
    for c in range(C):
        xs = np.zeros((SHARD_PAD, D_IN), dtype=np.float32)
        xs[: shard_sizes[c]] = x[starts[c]:ends[c]]
        xT_cores.append(np.ascontiguousarray(xs.T))
        dv = np.zeros(SHARD_PAD, dtype=np.float32)
        dv[: shard_sizes[c]] = dinv[starts[c]:ends[c]]
        dinv_cores.append(np.ascontiguousarray(dv.reshape(NB, P).T))
        mu = np.zeros(SHARD_PAD, dtype=np.float32)
        mu[: shard_sizes[c]] = mult[starts[c]:ends[c]]
        mult_cores.append(np.ascontiguousarray(mu.reshape(NB, P).T))

    cfg = dict(
        N=N, G=G, L=L, gpc=gpc, NB=NB, SHARD_PAD=SHARD_PAD, TAB_ROWS=TAB_ROWS,
        NSB=NSB, KD=KD, SLOT_DT=SLOT_DT, NMM=NMM, TOT=TOT, D_IN=D_IN,
        HT_W=HT_W, lo_u=lo_u.tolist(), wd_u=wd_u.tolist(),
        blocks_in_sb=blocks_in_sb,
        D_OUT=W2.shape[1], H1=W1.shape[1],
    )

    common = dict(
        W_emb=np.asarray(W_emb, np.float32),
        W_conv=np.asarray(W_conv, np.float32).reshape(L * HID, HID).astype(ml_dtypes.bfloat16),
        W1=np.asarray(W1, np.float32),
        W2=np.asarray(W2, np.float32),
        b_emb_b=np.tile(np.asarray(b_emb, np.float32), (P, 1)),
        b_conv_b=np.tile(
            np.asarray(b_conv, np.float32)[:, None, :], (1, P, 1)
        ).reshape(L * P, HID),
        b1_b=np.tile(np.asarray(b1, np.float32), (P, 1)),
        b2_b=np.tile(np.asarray(b2, np.float32), (P, 1)),
        iota=np.tile(np.arange(P, dtype=np.float32), (P, 1)).astype(
            ml_dtypes.bfloat16
        ),
        ident=np.eye(P, dtype=np.float32),
        identb=np.eye(P, dtype=np.float32).astype(ml_dtypes.bfloat16),
        cntinv=cntinv.reshape(G, 1),
    )
    per_core = [
        dict(
            xT=xT_cores[c], dinv_t=dinv_cores[c], idx=idx_cores[c],
            dstloc=dstloc_cores[c], gid=gid_cores[c], mult_t=mult_cores[c],
        )
        for c in range(C)
    ]
    return cfg, common, per_core


def _build(cfg):
    G, L = cfg["G"], cfg["L"]
    gpc, NB, SHARD_PAD = cfg["gpc"], cfg["NB"], cfg["SHARD_PAD"]
    TAB_ROWS, NSB, KD = cfg["TAB_ROWS"], cfg["NSB"], cfg["KD"]
    SLOT_DT, NMM, TOT = cfg["SLOT_DT"], cfg["NMM"], cfg["TOT"]
    D_IN, HT_W = cfg["D_IN"], cfg["HT_W"]
    blocks_in_sb = cfg["blocks_in_sb"]
    D_OUT, H1 = cfg["D_OUT"], cfg["H1"]
    H1H = H1 // 2
    WDMAX = int(max(cfg["wd_u"]))
    f32, bf16, i16 = mybir.dt.float32, mybir.dt.bfloat16, mybir.dt.int16
    AFT = mybir.ActivationFunctionType
    Alu = mybir.AluOpType

    nc = bacc.Bacc(
        "TRN2", target_bir_lowering=False, debug=False, num_devices=C,
        num_swdge_queues=4,
    )

    xT = nc.dram_tensor("xT", [D_IN, SHARD_PAD], f32, kind="ExternalInput")
    W_emb = nc.dram_tensor("W_emb", [D_IN, HID], f32, kind="ExternalInput")
    W_conv = nc.dram_tensor("W_conv", [L * HID, HID], bf16, kind="ExternalInput")
    W1 = nc.dram_tensor("W1", [3 * HID, H1], f32, kind="ExternalInput")
    W2 = nc.dram_tensor("W2", [H1, D_OUT], f32, kind="ExternalInput")
    b_emb_b = nc.dram_tensor("b_emb_b", [P, HID], f32, kind="ExternalInput")
    b_conv_b = nc.dram_tensor("b_conv_b", [L * P, HID], f32, kind="ExternalInput")
    b1_b = nc.dram_tensor("b1_b", [P, H1], f32, kind="ExternalInput")
    b2_b = nc.dram_tensor("b2_b", [P, D_OUT], f32, kind="ExternalInput")
    iota_d = nc.dram_tensor("iota", [P, P], bf16, kind="ExternalInput")
    ident_d = nc.dram_tensor("ident", [P, P], f32, kind="ExternalInput")
    identb_d = nc.dram_tensor("identb", [P, P], bf16, kind="ExternalInput")
    cntinv_d = nc.dram_tensor("cntinv", [G, 1], f32, kind="ExternalInput")
    dinv_d = nc.dram_tensor("dinv_t", [P, NB], f32, kind="ExternalInput")
    mult_d = nc.dram_tensor("mult_t", [P, NB], f32, kind="ExternalInput")
    idx_d = nc.dram_tensor("idx", [P, TOT // 16], i16, kind="ExternalInput")
    dstloc_d = nc.dram_tensor("dstloc", [P, NMM], bf16, kind="ExternalInput")
    gid_d = nc.dram_tensor("gid", [P, HT_W], bf16, kind="ExternalInput")
    out_d = nc.dram_tensor("out", [G, D_OUT], f32, kind="ExternalOutput")

    QROWS = SHARD_PAD // NTAB
    z_local = nc.dram_tensor("z_local", [SHARD_PAD, HID], bf16, kind="Internal")
    z_tabs = [
        [
            nc.dram_tensor(f"z_tab{t}_{i}", [C * QROWS, HID], bf16, kind="Internal")
            for i in range(2)
        ]
        for t in range(NTAB)
    ]
    pool_loc = nc.dram_tensor("pool_loc", [gpc, 2 * HID], f32, kind="Internal")
    pool_all = nc.dram_tensor("pool_all", [G, 2 * HID], f32, kind="Internal")
    dumps = (
        [
            nc.dram_tensor(f"hdump{i}", [P, SHARD_PAD], f32, kind="ExternalOutput")
            for i in range(L + 1)
        ]
        if DEBUG_DUMP
        else None
    )

    rg = [list(range(C))]

    with tile.TileContext(nc) as tc:
        with (
            tc.tile_pool(name="const", bufs=1) as cpool,
            tc.tile_pool(name="big", bufs=1) as bigpool,
            tc.tile_pool(name="g", bufs=14) as gpool,
            tc.tile_pool(name="s", bufs=2) as spool,
            tc.tile_pool(name="ix", bufs=12) as ixpool,
            tc.tile_pool(name="work", bufs=2) as wpool,
            tc.tile_pool(name="zst", bufs=2) as zpool,
            tc.tile_pool(name="zself", bufs=2) as zspool,
            tc.tile_pool(name="ps", bufs=3, space="PSUM") as pspool,
            tc.tile_pool(name="agg", bufs=5, space="PSUM") as aggpool,
        ):
            nc.gpsimd.load_library(library_config.mlp)

            def cload(dram_ap, shape, dtype, nm):
                t = cpool.tile(shape, dtype, name=nm, tag=nm)
                nc.sync.dma_start(t[:], dram_ap)
                return t

            Wemb_s = cload(W_emb[:], [D_IN, HID], f32, "Wemb_s")
            Wc_s = cload(
                W_conv[:].rearrange("(l k) h -> k l h", k=P), [P, L, HID], bf16
            , "Wc_s")
            W1_s = cload(W1[:].rearrange("(a k) h -> k a h", k=P), [P, 3, H1], f32, "W1_s")
            W2a_s = cload(W2[0:H1H, :], [H1H, D_OUT], f32, "W2a_s")
            W2b_s = cload(W2[H1H:H1, :], [H1H, D_OUT], f32, "W2b_s")
            bemb_s = cload(b_emb_b[:], [P, HID], f32, "bemb_s")
            mult_s = cload(mult_d[:], [P, NB], f32, "mult_s")
            bconv_s = cload(
                b_conv_b[:].rearrange("(l k) h -> k l h", k=P), [P, L, HID], f32
            , "bconv_s")
            b1_s = cload(b1_b[:], [P, H1], f32, "b1_s")
            b2_s = cload(b2_b[:], [P, D_OUT], f32, "b2_s")
            iota_s = cload(iota_d[:], [P, P], bf16, "iota_s")
            ident_s = cload(ident_d[:], [P, P], f32, "ident_s")
            identb_s = cload(identb_d[:], [P, P], bf16, "identb_s")
            cnt_s = cload(cntinv_d[:], [G, 1], f32, "cnt_s")
            dinv_s = cload(dinv_d[:], [P, NB], f32, "dinv_s")
            dstloc_s = cload(dstloc_d[:], [P, NMM], bf16, "dstloc_s")

            hbuf = bigpool.tile([P, SHARD_PAD], bf16, tag="h")

            # ---- helpers shared by embed and the layer loop ----
            QB = NB // NTAB            # blocks per quarter
            SBQ = QB // SBW            # superblocks per quarter

            def z_quarter(w_idx, qq):
                """z rows for quarter qq from hbuf via W_conv[w_idx]."""
                for b8 in range(qq * QB, (qq + 1) * QB, SBW):
                    zstage = zpool.tile([P, SBW * HID], bf16, tag="zst")
                    for j in range(SBW):
                        b = b8 + j
                        pst = pspool.tile([P, P], bf16, tag="ps")
                        nc.tensor.transpose(
                            out=pst[:], in_=hbuf[:, b * P:(b + 1) * P],
                            identity=identb_s[:],
                        )
                        hT_b = wpool.tile([P, P], bf16, tag="hTb")
                        nc.vector.tensor_copy(hT_b[:], pst[:])
                        psz = pspool.tile([P, HID], f32, tag="ps")
                        nc.tensor.matmul(
                            psz[:], lhsT=hT_b[:], rhs=Wc_s[:, w_idx, :],
                            start=True, stop=True,
                        )
                        nc.vector.tensor_copy(
                            zstage[:, j * HID:(j + 1) * HID], psz[:]
                        )
                    nc.sync.dma_start(
                        z_local[b8 * P:(b8 + SBW) * P, :].rearrange(
                            "(b p) h -> p b h", p=P
                        ),
                        zstage[:].rearrange("p (b h) -> p b h", h=HID),
                    )

            def ag_quarter(qq, parity):
                nc.gpsimd.collective_compute(
                    "AllGather", Alu.bypass,
                    replica_groups=rg,
                    ins=[z_local[qq * QROWS:(qq + 1) * QROWS, :].opt()],
                    outs=[z_tabs[qq][parity][:].opt()],
                )

            def epilogue_quarter(l, qq):
                """h_q = tanh(dinv*agg_q + b); if l<L-1 also *= dinv."""
                c0, c1 = qq * QB, (qq + 1) * QB
                hq = hbuf[:, c0 * P:c1 * P]
                dq = dinv_s[:, c0:c1]
                nc.vector.tensor_tensor(
                    out=hq.rearrange("p (b k) -> p b k", k=P),
                    in0=hq.rearrange("p (b k) -> p b k", k=P),
                    in1=dq.to_broadcast([P, QB, P]),
                    op=Alu.mult,
                )
                nc.vector.tensor_tensor(
                    out=hq.rearrange("p (b h) -> p b h", h=HID),
                    in0=hq.rearrange("p (b h) -> p b h", h=HID),
                    in1=bconv_s[:, l, :].rearrange(
                        "p (a h) -> p a h", a=1
                    ).to_broadcast([P, QB, HID]),
                    op=Alu.add,
                )
                nc.scalar.activation(hq, hq, AFT.Tanh)
                if l < L - 1:
                    nc.vector.tensor_tensor(
                        out=hq.rearrange("p (b k) -> p b k", k=P),
                        in0=hq.rearrange("p (b k) -> p b k", k=P),
                        in1=dq.to_broadcast([P, QB, P]),
                        op=Alu.mult,
                    )

            # ---- embed: h'0 = dinv * (x @ W_emb + b_emb), pipelined with
            # z(0) + its AllGather per completed quarter ----
            nextq = 0
            for b4 in range(0, NB, 4):
                nb4 = min(4, NB - b4)
                ps = pspool.tile([P, 4 * HID], f32, tag="ps")
                for j in range(nb4):
                    b = b4 + j
                    xt_b = wpool.tile([D_IN, P], f32, tag="xt")
                    nc.sync.dma_start(xt_b[:], xT[:, b * P:(b + 1) * P])
                    nc.tensor.matmul(
                        ps[:, j * HID:(j + 1) * HID],
                        lhsT=xt_b[:], rhs=Wemb_s[:],
                        start=True, stop=True,
                    )
                nc.vector.tensor_copy(
                    hbuf[:, b4 * P: b4 * P + nb4 * HID], ps[:, : nb4 * HID]
                )
                nc.vector.tensor_tensor(
                    out=hbuf[:, b4 * P: (b4 + nb4) * P].rearrange(
                        "p (b h) -> p b h", h=HID
                    ),
                    in0=hbuf[:, b4 * P: (b4 + nb4) * P].rearrange(
                        "p (b h) -> p b h", h=HID
                    ),
                    in1=bemb_s[:].rearrange(
                        "p (a h) -> p a h", a=1
                    ).to_broadcast([P, nb4, HID]),
                    op=Alu.add,
                )
                nc.vector.tensor_tensor(
                    out=hbuf[:, b4 * P: (b4 + nb4) * P].rearrange(
                        "p (b k) -> p b k", k=P
                    ),
                    in0=hbuf[:, b4 * P: (b4 + nb4) * P].rearrange(
                        "p (b k) -> p b k", k=P
                    ),
                    in1=dinv_s[:, b4: b4 + nb4].to_broadcast([P, nb4, P]),
                    op=Alu.mult,
                )
                while nextq < NTAB and (nextq + 1) * QB <= b4 + nb4:
                    z_quarter(0, nextq)
                    ag_quarter(nextq, 0)
                    nextq += 1
            if dumps is not None:
                nc.sync.dma_start(dumps[0][:], hbuf[:])

            # ---- layers ----
            for l in range(L):
                # AllGathers for layer l+1, deferred a few sbs so the pool
                # stream isn't head-of-line blocked on z being ready
                agq = []

                mcol = 0
                slot0 = 0
                for sb in range(NSB):
                    while agq and agq[0][0] <= sb:
                        _, qq = agq.pop(0)
                        ag_quarter(qq, (l + 1) % 2)
                    nblk = blocks_in_sb[sb]
                    aggs = [
                        aggpool.tile([P, HID], f32, tag="agg",
                                     name=f"agg_{l}_{sb}_{i}")
                        for i in range(nblk)
                    ]
                    sb_mcol = mcol
                    for ti, t in enumerate(range(NTAB)):
                        seg = nblk * SLOT_DT
                        t_slot0 = slot0 + t * seg
                        idxt = ixpool.tile(
                            [P, SBW * SLOT_DT // 16], i16, tag="ix"
                        )
                        nc.sync.dma_start(
                            idxt[:, : seg // 16],
                            idx_d[:, t_slot0 // 16:(t_slot0 + seg) // 16],
                        )
                        GC = GCHUNK
                        gtiles = []
                        off = 0
                        while off < seg:
                            n = min(GC, seg - off)
                            g = gpool.tile([P, GC // P, HID], bf16, tag="g")
                            nc.gpsimd.dma_gather(
                                g[:, : n // P, :],
                                z_tabs[t][l % 2][:],
                                idxt[:, off // 16:(off + n) // 16],
                                n, n, HID, single_packet=False,
                                queue_num=t,
                            )
                            gtiles.append(g)
                            off += n
                        m0 = sb_mcol + t * nblk * KD
                        sbt = spool.tile([P, SBW * KD * P], bf16, tag="s")
                        nc.vector.tensor_tensor(
                            out=sbt[:, : nblk * KD * P],
                            in0=dstloc_s[:, m0:m0 + nblk * KD].to_broadcast(
                                [P, nblk * KD, P]
                            ),
                            in1=iota_s[:].rearrange(
                                "p (a k) -> p a k", a=1
                            ).to_broadcast([P, nblk * KD, P]),
                            op=Alu.is_equal,
                        )
                        for di in range(nblk):
                            for kb in range(KD):
                                srel = (di * KD + kb) * P
                                ci, col = srel // GC, (srel % GC) // P
                                kk = di * KD + kb
                                nc.tensor.matmul(
                                    aggs[di][:],
                                    lhsT=sbt[:, kk * P:(kk + 1) * P],
                                    rhs=gtiles[ci][:, col, :],
                                    start=(ti == 0 and kb == 0),
                                    stop=(ti == NTAB - 1 and kb == KD - 1),
                                )
                    slot0 += NTAB * nblk * SLOT_DT
                    mcol += NTAB * nblk * KD
                    # self-loop contribution: hbuf_blk = agg + mult * z_local_blk
                    zs = zspool.tile([P, SBW * HID], bf16, tag="zs")
                    nc.sync.dma_start(
                        zs[:, : nblk * HID].rearrange("p (b h) -> p b h", h=HID),
                        z_local[
                            sb * SBW * P:(sb * SBW + nblk) * P, :
                        ].rearrange("(b p) h -> p b h", p=P),
                    )
                    zmul = zspool.tile([P, SBW * HID], bf16, tag="zmul")
                    for ai, a in enumerate(aggs):
                        b = sb * SBW + ai
                        nc.vector.tensor_scalar(
                            zmul[:, ai * HID:(ai + 1) * HID],
                            zs[:, ai * HID:(ai + 1) * HID],
                            mult_s[:, b:b + 1], None, Alu.mult,
                        )
                        nc.vector.tensor_tensor(
                            out=hbuf[:, b * P:(b + 1) * P],
                            in0=a[:],
                            in1=zmul[:, ai * HID:(ai + 1) * HID],
                            op=Alu.add,
                        )
                    # quarter finished → epilogue + next layer's z + deferred AG
                    if (sb + 1) % SBQ == 0:
                        qq = (sb + 1) // SBQ - 1
                        epilogue_quarter(l, qq)
                        if l < L - 1:
                            z_quarter(l + 1, qq)
                            agq.append((sb + 3, qq))
                for _, qq in agq:
                    ag_quarter(qq, (l + 1) % 2)
                if dumps is not None:
                    nc.sync.dma_start(dumps[l + 1][:], hbuf[:])

            # ---- pooling ----
            hT = bigpool.tile([P, HT_W], bf16, tag="hT")
            if HT_W > SHARD_PAD:
                nc.vector.memset(hT[:, SHARD_PAD:], 0.0)
            for b in range(NB):
                pst = pspool.tile([P, P], bf16, tag="ps")
                nc.tensor.transpose(
                    out=pst[:], in_=hbuf[:, b * P:(b + 1) * P],
                    identity=identb_s[:],
                )
                nc.vector.tensor_copy(hT[:, b * P:(b + 1) * P], pst[:])
            gid_s = bigpool.tile([P, HT_W], bf16, tag="gid")
            nc.sync.dma_start(gid_s[:], gid_d[:])

            sumP = wpool.tile([P, gpc], f32, tag="sumP")
            maxP = wpool.tile([P, gpc], f32, tag="maxP")
            for g in range(gpc):
                lo, wd = cfg["lo_u"][g], cfg["wd_u"][g]
                eq = wpool.tile([P, WDMAX], bf16, tag="eq")
                nc.vector.tensor_scalar(
                    eq[:, :wd], gid_s[:, lo:lo + wd], float(g), None,
                    Alu.is_equal,
                )
                msk = wpool.tile([P, WDMAX], f32, tag="msk")
                nc.vector.tensor_tensor(
                    out=msk[:, :wd], in0=hT[:, lo:lo + wd], in1=eq[:, :wd],
                    op=Alu.mult,
                )
                nc.vector.reduce_sum(
                    sumP[:, g:g + 1], msk[:, :wd], axis=mybir.AxisListType.X
                )
                nc.vector.tensor_scalar(
                    msk[:, :wd], eq[:, :wd], 60.0, -60.0, Alu.mult, Alu.add
                )
                nc.vector.tensor_tensor(
                    out=msk[:, :wd], in0=hT[:, lo:lo + wd], in1=msk[:, :wd],
                    op=Alu.add,
                )
                nc.vector.reduce_max(
                    maxP[:, g:g + 1], msk[:, :wd], axis=mybir.AxisListType.X
                )
            pg = pspool.tile([P, 2 * HID], f32, tag="ps")
            nc.tensor.transpose(
                out=pg[:gpc, :HID], in_=sumP[:], identity=ident_s[:]
            )
            nc.tensor.transpose(
                out=pg[:gpc, HID:], in_=maxP[:], identity=ident_s[:]
            )
            pl = wpool.tile([gpc, 2 * HID], f32, tag="pl")
            nc.vector.tensor_copy(pl[:], pg[:gpc, :])
            nc.sync.dma_start(pool_loc[:], pl[:])
            nc.gpsimd.collective_compute(
                "AllGather", Alu.bypass, replica_groups=rg,
                ins=[pool_loc[:].opt()], outs=[pool_all[:].opt()],
            )
            pa = wpool.tile([G, 2 * HID], f32, tag="pa")
            nc.sync.dma_start(pa[:], pool_all[:])
            mfix = wpool.tile([G, HID], f32, tag="mfix")
            nc.vector.tensor_scalar(
                mfix[:], pa[:, HID:], -50.0, None, Alu.is_gt
            )
            nc.vector.tensor_tensor(
                out=pa[:, HID:], in0=pa[:, HID:], in1=mfix[:], op=Alu.mult
            )
            mean_gf = wpool.tile([G, HID], f32, tag="mean")
            nc.vector.tensor_scalar(
                mean_gf[:], pa[:, :HID], cnt_s[:, :1], None, Alu.mult
            )
            gT = wpool.tile([P, 3 * G], f32, tag="gT")
            for a, src_ap in enumerate([pa[:, :HID], pa[:, HID:], mean_gf[:]]):
                ptx = pspool.tile([P, G], f32, tag="ps")
                nc.tensor.transpose(
                    out=ptx[:, :G], in_=src_ap, identity=ident_s[:G, :G]
                )
                nc.vector.tensor_copy(gT[:, a * G:(a + 1) * G], ptx[:, :G])

            # ---- head ----
            ph1 = pspool.tile([G, H1], f32, tag="ps")
            for a in range(3):
                nc.tensor.matmul(
                    ph1[:], lhsT=gT[:, a * G:(a + 1) * G], rhs=W1_s[:, a, :],
                    start=(a == 0), stop=(a == 2),
                )
            g1 = wpool.tile([G, H1], f32, tag="g1")
            nc.vector.tensor_tensor(
                out=g1[:], in0=ph1[:], in1=b1_s[:G, :], op=Alu.add
            )
            nc.scalar.activation(g1[:], g1[:], AFT.Lrelu, alpha=0.01)
            g1T = wpool.tile([H1H, 2 * G], f32, tag="g1T")
            for a in range(2):
                ptt = pspool.tile([H1H, G], f32, tag="ps")
                nc.tensor.transpose(
                    out=ptt[:], in_=g1[:, a * H1H:(a + 1) * H1H],
                    identity=ident_s[:G, :G],
                )
                nc.vector.tensor_copy(g1T[:, a * G:(a + 1) * G], ptt[:])
            ph2 = pspool.tile([G, D_OUT], f32, tag="ps")
            for a in range(2):
                nc.tensor.matmul(
                    ph2[:], lhsT=g1T[:, a * G:(a + 1) * G],
                    rhs=(W2a_s if a == 0 else W2b_s)[:],
                    start=(a == 0), stop=(a == 1),
                )
            go = wpool.tile([G, D_OUT], f32, tag="go")
            nc.vector.tensor_tensor(
                out=go[:], in0=ph2[:], in1=b2_s[:G, :], op=Alu.add
            )
            nc.scalar.activation(go[:], go[:], AFT.Lrelu, alpha=0.01)
            nc.sync.dma_start(out_d[:], go[:])

    nc.compile()
    return nc


def _install_ntff_shim():
    """Provide antenv.axon_hooks (missing in this image) so that
    run_bass_kernel_spmd(trace=True) can capture an NTFF profile via the
    injected libaxon_pjrt.so. Only used when TRACE=True."""
    import types
    import ctypes
    import contextlib

    try:
        from antenv.axon_hooks import get_axon_ntff_profile_hook  # noqa: F401
        return
    except ImportError:
        pass
    so_path = "/opt/axon/libaxon_pjrt.so"
    try:
        lib = ctypes.CDLL(so_path)
    except OSError:
        return
    if not hasattr(lib, "axon_start_nrt_profile"):
        return
    lib.axon_start_nrt_profile.argtypes = [
        ctypes.POINTER(ctypes.c_int64), ctypes.c_size_t,
    ]
    lib.axon_start_nrt_profile.restype = ctypes.c_int64
    lib.axon_stop_nrt_profile.argtypes = [ctypes.c_char_p]
    lib.axon_stop_nrt_profile.restype = ctypes.c_int64

    @contextlib.contextmanager
    def _hook(output_dir, device_ids):
        import jax
        jax.devices()
        if device_ids:
            ids = (ctypes.c_int64 * len(device_ids))(*device_ids)
            rc = lib.axon_start_nrt_profile(ids, len(device_ids))
        else:
            rc = lib.axon_start_nrt_profile(None, 0)
        if rc != 0:
            raise RuntimeError(f"axon_start_nrt_profile rc={rc}")
        try:
            yield
        finally:
            n = lib.axon_stop_nrt_profile(str(output_dir).encode())
            print(f"profile: {n} file(s) written to {output_dir}",
                  file=sys.stderr)

    mod = types.ModuleType("antenv.axon_hooks")
    mod.get_axon_ntff_profile_hook = lambda: _hook
    mod.set_axon_ntff_profile_hook = lambda h: None
    sys.modules["antenv.axon_hooks"] = mod


def kernel(**inputs):
    global LAST_RESULTS
    if TRACE:
        _install_ntff_shim()
    ins = {k: np.asarray(v) for k, v in inputs.items()}
    cfg, common, per_core = _host_prep(
        ins["x"].astype(np.float32), ins["edge_index"], ins["batch"],
        ins["W_emb"], ins["b_emb"], ins["W_conv"], ins["b_conv"],
        ins["W1"], ins["b1"], ins["W2"], ins["b2"],
    )
    nc = _build(cfg)

    in_maps = []
    for c in range(C):
        m = dict(
            xT=per_core[c]["xT"],
            W_emb=common["W_emb"], W_conv=common["W_conv"],
            W1=common["W1"], W2=common["W2"],
            b_emb_b=common["b_emb_b"], b_conv_b=common["b_conv_b"],
            b1_b=common["b1_b"], b2_b=common["b2_b"],
            iota=common["iota"], ident=common["ident"],
            identb=common["identb"],
            cntinv=common["cntinv"],
            dinv_t=per_core[c]["dinv_t"], idx=per_core[c]["idx"],
            dstloc=per_core[c]["dstloc"], gid=per_core[c]["gid"],
            mult_t=per_core[c]["mult_t"],
        )
        in_maps.append(m)

    res = run_bass_kernel_spmd(
        nc, in_maps, core_ids=list(range(C)), trace=TRACE
    )
    LAST_RESULTS = res
    return np.asarray(res.results[0]["out"], dtype=np.float32)



# revision 37
# speedup vs baseline: 1.0112x; 1.0013x over previous
"""GCN message-passing kernel for Trainium2 (8 NeuronCores).

Strategy:
  - Nodes sharded across 8 cores, aligned to graph boundaries (G/8 graphs/core).
  - Edges partitioned by destination shard; per layer each core computes
    z = (dinv * h) @ W for its shard, the z-table is AllGathered (bf16),
    source rows are fetched with dma_gather (4 int16-indexed sub-tables),
    and a one-hot scatter-matmul accumulates messages per 128-dst block.
  - The schedule is padded to be identical on all cores (single SPMD program).
  - Graph pooling: masked static-window reduces on h^T + one AllGather.
"""

import sys

sys.path.insert(0, "/opt/trn_rl_repo")

import numpy as np
import ml_dtypes

import concourse.bass as bass
import concourse.bacc as bacc
import concourse.tile as tile
from concourse import mybir, library_config
from concourse.bass_utils import run_bass_kernel_spmd

C = 8            # cores
P = 128          # partitions / block size
HID = 128
SBW = 5          # dst blocks per superblock
GCHUNK = 2048    # max idxs per dma_gather call
NTAB = 4         # gather sub-tables (one per shard quarter)

LAST_RESULTS = None  # set by kernel(): BassKernelResults of the last run
TRACE = False        # set True (e.g. by test.py) to capture an NTFF profile
DEBUG_DUMP = False   # dump per-layer h buffers as extra outputs


def _host_prep(x, edge_index, batch, W_emb, b_emb, W_conv, b_conv, W1, b1, W2, b2):
    N = x.shape[0]
    batch = np.asarray(batch, dtype=np.int64)
    G = int(batch.max()) + 1
    assert G % C == 0, G
    L = W_conv.shape[0]

    src = np.asarray(edge_index[0], dtype=np.int64)
    dst = np.asarray(edge_index[1], dtype=np.int64)
    self_idx = np.arange(N, dtype=np.int64)
    src = np.concatenate([src, self_idx])
    dst = np.concatenate([dst, self_idx])

    deg = np.bincount(dst, minlength=N).astype(np.float64)
    dinv = (1.0 / np.sqrt(np.maximum(deg, 1e-12))).astype(np.float32)
    dinv[deg <= 0] = 0.0

    # self-loop edges (natural + the appended ones) are handled as an
    # elementwise z_local add with per-node multiplicity, not as gathers
    keep = src != dst
    mult = np.bincount(dst[~keep], minlength=N).astype(np.float32)
    src = src[keep]
    dst = dst[keep]

    # graph-aligned sharding: core c owns graphs [c*G/C, (c+1)*G/C)
    gpc = G // C
    bounds = np.searchsorted(batch, np.arange(G + 1))
    starts = bounds[np.arange(C) * gpc]
    ends = bounds[(np.arange(C) + 1) * gpc]
    shard_sizes = ends - starts
    NB = int(np.ceil(shard_sizes.max() / P))
    # round NB up to a multiple of SBW so superblocks tile quarters evenly
    NB = int(np.ceil(NB / (4 * SBW)) * 4 * SBW)
    SHARD_PAD = NB * P
    QROWS = SHARD_PAD // NTAB
    TAB_ROWS = C * QROWS
    assert TAB_ROWS <= 32767, TAB_ROWS

    core_of = np.searchsorted(ends - 1, np.arange(N), side="left")
    loc_row = np.arange(N) - starts[core_of]
    e_tab = (loc_row // QROWS)[src].astype(np.int64)
    e_trow = (core_of * QROWS + (loc_row % QROWS))[src].astype(np.int64)
    e_core = core_of[dst]
    e_dloc = dst - starts[e_core]

    NSB = int(np.ceil(NB / SBW))

    dblk = e_dloc // P
    key = ((e_core * NB + dblk) * NTAB + e_tab).astype(np.int64)
    cnts = np.bincount(key, minlength=C * NB * NTAB)
    KD = int(np.ceil(cnts.max() / P))
    SLOT_DT = KD * P
    NMM = NB * NTAB * KD
    TOT = NMM * P

    # schedule order per core: (sb, tab, d in sb, kb)
    blocks_in_sb = [min(NB - s * SBW, SBW) for s in range(NSB)]
    grank = np.zeros((NB, NTAB), dtype=np.int64)
    acc = 0
    for s in range(NSB):
        nblk = blocks_in_sb[s]
        for t in range(NTAB):
            for j in range(nblk):
                grank[s * SBW + j, t] = acc + t * nblk + j
        acc += NTAB * nblk
    NGRP = acc
    assert NGRP == NB * NTAB

    idx_cores, dstloc_cores = [], []
    for c in range(C):
        m = e_core == c
        tab_c = e_tab[m]
        trow_c = e_trow[m]
        dloc_c = e_dloc[m]
        gr = grank[dloc_c // P, tab_c]
        order = np.argsort(gr, kind="stable")
        gr_s = gr[order]
        grp_first = np.searchsorted(gr_s, np.arange(NGRP))
        grp_cnt = np.diff(np.append(grp_first, len(gr_s)))
        assert grp_cnt.max() <= SLOT_DT, (grp_cnt.max(), SLOT_DT)
        within = np.arange(len(gr_s)) - grp_first[gr_s]
        slot = gr_s * SLOT_DT + within

        idx_arr = np.zeros(TOT, dtype=np.int16)
        idx_arr[slot] = trow_c[order].astype(np.int16)
        dl_arr = np.full(TOT, -1.0, dtype=np.float32)
        dl_arr[slot] = (dloc_c[order] % P).astype(np.float32)

        idx_wrapped = np.tile(
            np.ascontiguousarray(idx_arr.reshape(-1, 16).T), (C, 1)
        )
        dstloc = np.ascontiguousarray(
            dl_arr.reshape(NMM, P).T.astype(ml_dtypes.bfloat16)
        )
        idx_cores.append(idx_wrapped)
        dstloc_cores.append(dstloc)

    # pooling windows: static lo/width per local graph, masked by gid data
    lo_u = np.zeros(gpc, dtype=np.int64)
    wd_u = np.zeros(gpc, dtype=np.int64)
    for g in range(gpc):
        los = bounds[np.arange(C) * gpc + g] - starts
        his = bounds[np.arange(C) * gpc + g + 1] - starts
        lo_u[g] = los.min()
        wd_u[g] = max(his.max() - lo_u[g], 1)
    HT_W = int(max(SHARD_PAD, (lo_u + wd_u).max()))

    gid_cores = []
    for c in range(C):
        gid = np.full(HT_W, -1.0, dtype=np.float32)
        n = shard_sizes[c]
        gid[:n] = (batch[starts[c]:ends[c]] - c * gpc).astype(np.float32)
        gid_cores.append(
            np.ascontiguousarray(np.tile(gid.astype(ml_dtypes.bfloat16), (P, 1)))
        )

    cnt = (bounds[1:] - bounds[:-1]).astype(np.float32)
    cntinv = (1.0 / np.maximum(cnt, 1.0)).astype(np.float32)

    xT_cores, dinv_cores, mult_cores = [], [], []
    D_IN = x.shape[1]
    for c in range(C):
        xs = np.zeros((SHARD_PAD, D_IN), dtype=np.float32)
        xs[: shard_sizes[c]] = x[starts[c]:ends[c]]
        xT_cores.append(np.ascontiguousarray(xs.T))
        dv = np.zeros(SHARD_PAD, dtype=np.float32)
        dv[: shard_sizes[c]] = dinv[starts[c]:ends[c]]
        dinv_cores.append(np.ascontiguousarray(dv.reshape(NB, P).T))
        mu = np.zeros(SHARD_PAD, dtype=np.float32)
        mu[: shard_sizes[c]] = mult[starts[c]:ends[c]]
        mult_cores.append(np.ascontiguousarray(mu.reshape(NB, P).T))

    cfg = dict(
        N=N, G=G, L=L, gpc=gpc, NB=NB, SHARD_PAD=SHARD_PAD, TAB_ROWS=TAB_ROWS,
        NSB=NSB, KD=KD, SLOT_DT=SLOT_DT, NMM=NMM, TOT=TOT, D_IN=D_IN,
        HT_W=HT_W, lo_u=lo_u.tolist(), wd_u=wd_u.tolist(),
        blocks_in_sb=blocks_in_sb,
        D_OUT=W2.shape[1], H1=W1.shape[1],
    )

    common = dict(
        W_emb=np.asarray(W_emb, np.float32),
        W_conv=np.asarray(W_conv, np.float32).reshape(L * HID, HID).astype(ml_dtypes.bfloat16),
        W1=np.asarray(W1, np.float32),
        W2=np.asarray(W2, np.float32),
        b_emb_b=np.tile(np.asarray(b_emb, np.float32), (P, 1)),
        b_conv_b=np.tile(
            np.asarray(b_conv, np.float32)[:, None, :], (1, P, 1)
        ).reshape(L * P, HID),
        b1_b=np.tile(np.asarray(b1, np.float32), (P, 1)),
        b2_b=np.tile(np.asarray(b2, np.float32), (P, 1)),
        iota=np.tile(np.arange(P, dtype=np.float32), (P, 1)).astype(
            ml_dtypes.bfloat16
        ),
        ident=np.eye(P, dtype=np.float32),
        identb=np.eye(P, dtype=np.float32).astype(ml_dtypes.bfloat16),
        cntinv=cntinv.reshape(G, 1),
    )
    per_core = [
        dict(
            xT=xT_cores[c], dinv_t=dinv_cores[c], idx=idx_cores[c],
            dstloc=dstloc_cores[c], gid=gid_cores[c], mult_t=mult_cores[c],
        )
        for c in range(C)
    ]
    return cfg, common, per_core


def _build(cfg):
    G, L = cfg["G"], cfg["L"]
    gpc, NB, SHARD_PAD = cfg["gpc"], cfg["NB"], cfg["SHARD_PAD"]
    TAB_ROWS, NSB, KD = cfg["TAB_ROWS"], cfg["NSB"], cfg["KD"]
    SLOT_DT, NMM, TOT = cfg["SLOT_DT"], cfg["NMM"], cfg["TOT"]
    D_IN, HT_W = cfg["D_IN"], cfg["HT_W"]
    blocks_in_sb = cfg["blocks_in_sb"]
    D_OUT, H1 = cfg["D_OUT"], cfg["H1"]
    H1H = H1 // 2
    WDMAX = int(max(cfg["wd_u"]))
    f32, bf16, i16 = mybir.dt.float32, mybir.dt.bfloat16, mybir.dt.int16
    AFT = mybir.ActivationFunctionType
    Alu = mybir.AluOpType

    nc = bacc.Bacc(
        "TRN2", target_bir_lowering=False, debug=False, num_devices=C,
        num_swdge_queues=4,
    )

    xT = nc.dram_tensor("xT", [D_IN, SHARD_PAD], f32, kind="ExternalInput")
    W_emb = nc.dram_tensor("W_emb", [D_IN, HID], f32, kind="ExternalInput")
    W_conv = nc.dram_tensor("W_conv", [L * HID, HID], bf16, kind="ExternalInput")
    W1 = nc.dram_tensor("W1", [3 * HID, H1], f32, kind="ExternalInput")
    W2 = nc.dram_tensor("W2", [H1, D_OUT], f32, kind="ExternalInput")
    b_emb_b = nc.dram_tensor("b_emb_b", [P, HID], f32, kind="ExternalInput")
    b_conv_b = nc.dram_tensor("b_conv_b", [L * P, HID], f32, kind="ExternalInput")
    b1_b = nc.dram_tensor("b1_b", [P, H1], f32, kind="ExternalInput")
    b2_b = nc.dram_tensor("b2_b", [P, D_OUT], f32, kind="ExternalInput")
    iota_d = nc.dram_tensor("iota", [P, P], bf16, kind="ExternalInput")
    ident_d = nc.dram_tensor("ident", [P, P], f32, kind="ExternalInput")
    identb_d = nc.dram_tensor("identb", [P, P], bf16, kind="ExternalInput")
    cntinv_d = nc.dram_tensor("cntinv", [G, 1], f32, kind="ExternalInput")
    dinv_d = nc.dram_tensor("dinv_t", [P, NB], f32, kind="ExternalInput")
    mult_d = nc.dram_tensor("mult_t", [P, NB], f32, kind="ExternalInput")
    idx_d = nc.dram_tensor("idx", [P, TOT // 16], i16, kind="ExternalInput")
    dstloc_d = nc.dram_tensor("dstloc", [P, NMM], bf16, kind="ExternalInput")
    gid_d = nc.dram_tensor("gid", [P, HT_W], bf16, kind="ExternalInput")
    out_d = nc.dram_tensor("out", [G, D_OUT], f32, kind="ExternalOutput")

    QROWS = SHARD_PAD // NTAB
    z_local = nc.dram_tensor("z_local", [SHARD_PAD, HID], bf16, kind="Internal")
    z_tabs = [
        [
            nc.dram_tensor(f"z_tab{t}_{i}", [C * QROWS, HID], bf16, kind="Internal")
            for i in range(2)
        ]
        for t in range(NTAB)
    ]
    pool_loc = nc.dram_tensor("pool_loc", [gpc, 2 * HID], f32, kind="Internal")
    pool_all = nc.dram_tensor("pool_all", [G, 2 * HID], f32, kind="Internal")
    dumps = (
        [
            nc.dram_tensor(f"hdump{i}", [P, SHARD_PAD], f32, kind="ExternalOutput")
            for i in range(L + 1)
        ]
        if DEBUG_DUMP
        else None
    )

    rg = [list(range(C))]

    with tile.TileContext(nc) as tc:
        with (
            tc.tile_pool(name="const", bufs=1) as cpool,
            tc.tile_pool(name="big", bufs=1) as bigpool,
            tc.tile_pool(name="g", bufs=14) as gpool,
            tc.tile_pool(name="s", bufs=2) as spool,
            tc.tile_pool(name="ix", bufs=12) as ixpool,
            tc.tile_pool(name="work", bufs=2) as wpool,
            tc.tile_pool(name="zst", bufs=2) as zpool,
            tc.tile_pool(name="zself", bufs=2) as zspool,
            tc.tile_pool(name="ps", bufs=3, space="PSUM") as pspool,
            tc.tile_pool(name="agg", bufs=5, space="PSUM") as aggpool,
        ):
            nc.gpsimd.load_library(library_config.mlp)

            def cload(dram_ap, shape, dtype, nm):
                t = cpool.tile(shape, dtype, name=nm, tag=nm)
                nc.sync.dma_start(t[:], dram_ap)
                return t

            Wemb_s = cload(W_emb[:], [D_IN, HID], f32, "Wemb_s")
            Wc_s = cload(
                W_conv[:].rearrange("(l k) h -> k l h", k=P), [P, L, HID], bf16
            , "Wc_s")
            W1_s = cload(W1[:].rearrange("(a k) h -> k a h", k=P), [P, 3, H1], f32, "W1_s")
            W2a_s = cload(W2[0:H1H, :], [H1H, D_OUT], f32, "W2a_s")
            W2b_s = cload(W2[H1H:H1, :], [H1H, D_OUT], f32, "W2b_s")
            bemb_s = cload(b_emb_b[:], [P, HID], f32, "bemb_s")
            mult_s = cload(mult_d[:], [P, NB], f32, "mult_s")
            bconv_s = cload(
                b_conv_b[:].rearrange("(l k) h -> k l h", k=P), [P, L, HID], f32
            , "bconv_s")
            b1_s = cload(b1_b[:], [P, H1], f32, "b1_s")
            b2_s = cload(b2_b[:], [P, D_OUT], f32, "b2_s")
            iota_s = cload(iota_d[:], [P, P], bf16, "iota_s")
            ident_s = cload(ident_d[:], [P, P], f32, "ident_s")
            identb_s = cload(identb_d[:], [P, P], bf16, "identb_s")
            cnt_s = cload(cntinv_d[:], [G, 1], f32, "cnt_s")
            dinv_s = cload(dinv_d[:], [P, NB], f32, "dinv_s")
            dstloc_s = cload(dstloc_d[:], [P, NMM], bf16, "dstloc_s")

            hbuf = bigpool.tile([P, SHARD_PAD], bf16, tag="h")

            # ---- helpers shared by embed and the layer loop ----
            QB = NB // NTAB            # blocks per quarter
            SBQ = QB // SBW            # superblocks per quarter

            def z_quarter(w_idx, qq):
                """z rows for quarter qq from hbuf via W_conv[w_idx]."""
                for b8 in range(qq * QB, (qq + 1) * QB, SBW):
                    zstage = zpool.tile([P, SBW * HID], bf16, tag="zst")
                    for j in range(SBW):
                        b = b8 + j
                        pst = pspool.tile([P, P], bf16, tag="ps")
                        nc.tensor.transpose(
                            out=pst[:], in_=hbuf[:, b * P:(b + 1) * P],
                            identity=identb_s[:],
                        )
                        hT_b = wpool.tile([P, P], bf16, tag="hTb")
                        nc.vector.tensor_copy(hT_b[:], pst[:])
                        psz = pspool.tile([P, HID], f32, tag="ps")
                        nc.tensor.matmul(
                            psz[:], lhsT=hT_b[:], rhs=Wc_s[:, w_idx, :],
                            start=True, stop=True,
                        )
                        nc.vector.tensor_copy(
                            zstage[:, j * HID:(j + 1) * HID], psz[:]
                        )
                    nc.sync.dma_start(
                        z_local[b8 * P:(b8 + SBW) * P, :].rearrange(
                            "(b p) h -> p b h", p=P
                        ),
                        zstage[:].rearrange("p (b h) -> p b h", h=HID),
                    )

            def ag_quarter(qq, parity):
                nc.gpsimd.collective_compute(
                    "AllGather", Alu.bypass,
                    replica_groups=rg,
                    ins=[z_local[qq * QROWS:(qq + 1) * QROWS, :].opt()],
                    outs=[z_tabs[qq][parity][:].opt()],
                )

            def epilogue_quarter(l, qq):
                """h_q = tanh(dinv*agg_q + b); if l<L-1 also *= dinv."""
                c0, c1 = qq * QB, (qq + 1) * QB
                hq = hbuf[:, c0 * P:c1 * P]
                dq = dinv_s[:, c0:c1]
                nc.vector.tensor_tensor(
                    out=hq.rearrange("p (b k) -> p b k", k=P),
                    in0=hq.rearrange("p (b k) -> p b k", k=P),
                    in1=dq.to_broadcast([P, QB, P]),
                    op=Alu.mult,
                )
                nc.vector.tensor_tensor(
                    out=hq.rearrange("p (b h) -> p b h", h=HID),
                    in0=hq.rearrange("p (b h) -> p b h", h=HID),
                    in1=bconv_s[:, l, :].rearrange(
                        "p (a h) -> p a h", a=1
                    ).to_broadcast([P, QB, HID]),
                    op=Alu.add,
                )
                nc.scalar.activation(hq, hq, AFT.Tanh)
                if l < L - 1:
                    nc.vector.tensor_tensor(
                        out=hq.rearrange("p (b k) -> p b k", k=P),
                        in0=hq.rearrange("p (b k) -> p b k", k=P),
                        in1=dq.to_broadcast([P, QB, P]),
                        op=Alu.mult,
                    )

            # ---- embed: h'0 = dinv * (x @ W_emb + b_emb), pipelined with
            # z(0) + its AllGather per completed quarter ----
            nextq = 0
            for b4 in range(0, NB, 4):
                nb4 = min(4, NB - b4)
                ps = pspool.tile([P, 4 * HID], f32, tag="ps")
                for j in range(nb4):
                    b = b4 + j
                    xt_b = wpool.tile([D_IN, P], f32, tag="xt")
                    nc.sync.dma_start(xt_b[:], xT[:, b * P:(b + 1) * P])
                    nc.tensor.matmul(
                        ps[:, j * HID:(j + 1) * HID],
                        lhsT=xt_b[:], rhs=Wemb_s[:],
                        start=True, stop=True,
                    )
                nc.vector.tensor_copy(
                    hbuf[:, b4 * P: b4 * P + nb4 * HID], ps[:, : nb4 * HID]
                )
                nc.vector.tensor_tensor(
                    out=hbuf[:, b4 * P: (b4 + nb4) * P].rearrange(
                        "p (b h) -> p b h", h=HID
                    ),
                    in0=hbuf[:, b4 * P: (b4 + nb4) * P].rearrange(
                        "p (b h) -> p b h", h=HID
                    ),
                    in1=bemb_s[:].rearrange(
                        "p (a h) -> p a h", a=1
                    ).to_broadcast([P, nb4, HID]),
                    op=Alu.add,
                )
                nc.vector.tensor_tensor(
                    out=hbuf[:, b4 * P: (b4 + nb4) * P].rearrange(
                        "p (b k) -> p b k", k=P
                    ),
                    in0=hbuf[:, b4 * P: (b4 + nb4) * P].rearrange(
                        "p (b k) -> p b k", k=P
                    ),
                    in1=dinv_s[:, b4: b4 + nb4].to_broadcast([P, nb4, P]),
                    op=Alu.mult,
                )
                while nextq < NTAB and (nextq + 1) * QB <= b4 + nb4:
                    z_quarter(0, nextq)
                    ag_quarter(nextq, 0)
                    nextq += 1
            if dumps is not None:
                nc.sync.dma_start(dumps[0][:], hbuf[:])

            # ---- layers ----
            for l in range(L):
                # AllGathers for layer l+1, deferred a few sbs so the pool
                # stream isn't head-of-line blocked on z being ready
                agq = []

                mcol = 0
                slot0 = 0
                for sb in range(NSB):
                    while agq and agq[0][0] <= sb:
                        _, qq = agq.pop(0)
                        ag_quarter(qq, (l + 1) % 2)
                    nblk = blocks_in_sb[sb]
                    aggs = [
                        aggpool.tile([P, HID], f32, tag="agg",
                                     name=f"agg_{l}_{sb}_{i}")
                        for i in range(nblk)
                    ]
                    sb_mcol = mcol
                    for ti, t in enumerate(range(NTAB)):
                        seg = nblk * SLOT_DT
                        t_slot0 = slot0 + t * seg
                        idxt = ixpool.tile(
                            [P, SBW * SLOT_DT // 16], i16, tag="ix"
                        )
                        nc.sync.dma_start(
                            idxt[:, : seg // 16],
                            idx_d[:, t_slot0 // 16:(t_slot0 + seg) // 16],
                        )
                        GC = GCHUNK
                        gtiles = []
                        off = 0
                        while off < seg:
                            n = min(GC, seg - off)
                            g = gpool.tile([P, GC // P, HID], bf16, tag="g")
                            nc.gpsimd.dma_gather(
                                g[:, : n // P, :],
                                z_tabs[t][l % 2][:],
                                idxt[:, off // 16:(off + n) // 16],
                                n, n, HID, single_packet=False,
                                queue_num=t,
                            )
                            gtiles.append(g)
                            off += n
                        m0 = sb_mcol + t * nblk * KD
                        sbt = spool.tile([P, SBW * KD * P], bf16, tag="s")
                        nc.vector.tensor_tensor(
                            out=sbt[:, : nblk * KD * P],
                            in0=dstloc_s[:, m0:m0 + nblk * KD].to_broadcast(
                                [P, nblk * KD, P]
                            ),
                            in1=iota_s[:].rearrange(
                                "p (a k) -> p a k", a=1
                            ).to_broadcast([P, nblk * KD, P]),
                            op=Alu.is_equal,
                        )
                        for di in range(nblk):
                            for kb in range(KD):
                                srel = (di * KD + kb) * P
                                ci, col = srel // GC, (srel % GC) // P
                                kk = di * KD + kb
                                nc.tensor.matmul(
                                    aggs[di][:],
                                    lhsT=sbt[:, kk * P:(kk + 1) * P],
                                    rhs=gtiles[ci][:, col, :],
                                    start=(ti == 0 and kb == 0),
                                    stop=(ti == NTAB - 1 and kb == KD - 1),
                                )
                    slot0 += NTAB * nblk * SLOT_DT
                    mcol += NTAB * nblk * KD
                    # self-loop contribution: hbuf_blk = agg + mult * z_local_blk
                    zs = zspool.tile([P, SBW * HID], bf16, tag="zs")
                    nc.sync.dma_start(
                        zs[:, : nblk * HID].rearrange("p (b h) -> p b h", h=HID),
                        z_local[
                            sb * SBW * P:(sb * SBW + nblk) * P, :
                        ].rearrange("(b p) h -> p b h", p=P),
                    )
                    zmul = zspool.tile([P, SBW * HID], bf16, tag="zmul")
                    for ai, a in enumerate(aggs):
                        b = sb * SBW + ai
                        nc.vector.tensor_scalar(
                            zmul[:, ai * HID:(ai + 1) * HID],
                            zs[:, ai * HID:(ai + 1) * HID],
                            mult_s[:, b:b + 1], None, Alu.mult,
                        )
                        nc.vector.tensor_tensor(
                            out=hbuf[:, b * P:(b + 1) * P],
                            in0=a[:],
                            in1=zmul[:, ai * HID:(ai + 1) * HID],
                            op=Alu.add,
                        )
                    # quarter finished → epilogue + next layer's z + deferred AG
                    if (sb + 1) % SBQ == 0:
                        qq = (sb + 1) // SBQ - 1
                        epilogue_quarter(l, qq)
                        if l < L - 1:
                            z_quarter(l + 1, qq)
                            agq.append((sb + 3, qq))
                for _, qq in agq:
                    ag_quarter(qq, (l + 1) % 2)
                if dumps is not None:
                    nc.sync.dma_start(dumps[l + 1][:], hbuf[:])

            # ---- pooling ----
            hT = bigpool.tile([P, HT_W], bf16, tag="hT")
            if HT_W > SHARD_PAD:
                nc.vector.memset(hT[:, SHARD_PAD:], 0.0)
            for b in range(NB):
                pst = pspool.tile([P, P], bf16, tag="ps")
                nc.tensor.transpose(
                    out=pst[:], in_=hbuf[:, b * P:(b + 1) * P],
                    identity=identb_s[:],
                )
                nc.vector.tensor_copy(hT[:, b * P:(b + 1) * P], pst[:])
            gid_s = bigpool.tile([P, HT_W], bf16, tag="gid")
            nc.sync.dma_start(gid_s[:], gid_d[:])

            sumP = wpool.tile([P, gpc], f32, tag="sumP")
            maxP = wpool.tile([P, gpc], f32, tag="maxP")
            for g in range(gpc):
                lo, wd = cfg["lo_u"][g], cfg["wd_u"][g]
                eq = wpool.tile([P, WDMAX], bf16, tag="eq")
                nc.vector.tensor_scalar(
                    eq[:, :wd], gid_s[:, lo:lo + wd], float(g), None,
                    Alu.is_equal,
                )
                msk = wpool.tile([P, WDMAX], f32, tag="msk")
                nc.vector.tensor_tensor(
                    out=msk[:, :wd], in0=hT[:, lo:lo + wd], in1=eq[:, :wd],
                    op=Alu.mult,
                )
                nc.vector.reduce_sum(
                    sumP[:, g:g + 1], msk[:, :wd], axis=mybir.AxisListType.X
                )
                nc.vector.tensor_scalar(
                    msk[:, :wd], eq[:, :wd], 60.0, -60.0, Alu.mult, Alu.add
                )
                nc.vector.tensor_tensor(
                    out=msk[:, :wd], in0=hT[:, lo:lo + wd], in1=msk[:, :wd],
                    op=Alu.add,
                )
                nc.vector.reduce_max(
                    maxP[:, g:g + 1], msk[:, :wd], axis=mybir.AxisListType.X
                )
            pg = pspool.tile([P, 2 * HID], f32, tag="ps")
            nc.tensor.transpose(
                out=pg[:gpc, :HID], in_=sumP[:], identity=ident_s[:]
            )
            nc.tensor.transpose(
                out=pg[:gpc, HID:], in_=maxP[:], identity=ident_s[:]
            )
            pl = wpool.tile([gpc, 2 * HID], f32, tag="pl")
            nc.vector.tensor_copy(pl[:], pg[:gpc, :])
            nc.sync.dma_start(pool_loc[:], pl[:])
            nc.gpsimd.collective_compute(
                "AllGather", Alu.bypass, replica_groups=rg,
                ins=[pool_loc[:].opt()], outs=[pool_all[:].opt()],
            )
            pa = wpool.tile([G, 2 * HID], f32, tag="pa")
            nc.sync.dma_start(pa[:], pool_all[:])
            mfix = wpool.tile([G, HID], f32, tag="mfix")
            nc.vector.tensor_scalar(
                mfix[:], pa[:, HID:], -50.0, None, Alu.is_gt
            )
            nc.vector.tensor_tensor(
                out=pa[:, HID:], in0=pa[:, HID:], in1=mfix[:], op=Alu.mult
            )
            mean_gf = wpool.tile([G, HID], f32, tag="mean")
            nc.vector.tensor_scalar(
                mean_gf[:], pa[:, :HID], cnt_s[:, :1], None, Alu.mult
            )
            gT = wpool.tile([P, 3 * G], f32, tag="gT")
            for a, src_ap in enumerate([pa[:, :HID], pa[:, HID:], mean_gf[:]]):
                ptx = pspool.tile([P, G], f32, tag="ps")
                nc.tensor.transpose(
                    out=ptx[:, :G], in_=src_ap, identity=ident_s[:G, :G]
                )
                nc.vector.tensor_copy(gT[:, a * G:(a + 1) * G], ptx[:, :G])

            # ---- head ----
            ph1 = pspool.tile([G, H1], f32, tag="ps")
            for a in range(3):
                nc.tensor.matmul(
                    ph1[:], lhsT=gT[:, a * G:(a + 1) * G], rhs=W1_s[:, a, :],
                    start=(a == 0), stop=(a == 2),
                )
            g1 = wpool.tile([G, H1], f32, tag="g1")
            nc.vector.tensor_tensor(
                out=g1[:], in0=ph1[:], in1=b1_s[:G, :], op=Alu.add
            )
            nc.scalar.activation(g1[:], g1[:], AFT.Lrelu, alpha=0.01)
            g1T = wpool.tile([H1H, 2 * G], f32, tag="g1T")
            for a in range(2):
                ptt = pspool.tile([H1H, G], f32, tag="ps")
                nc.tensor.transpose(
                    out=ptt[:], in_=g1[:, a * H1H:(a + 1) * H1H],
                    identity=ident_s[:G, :G],
                )
                nc.vector.tensor_copy(g1T[:, a * G:(a + 1) * G], ptt[:])
            ph2 = pspool.tile([G, D_OUT], f32, tag="ps")
            for a in range(2):
                nc.tensor.matmul(
                    ph2[:], lhsT=g1T[:, a * G:(a + 1) * G],
                    rhs=(W2a_s if a == 0 else W2b_s)[:],
                    start=(a == 0), stop=(a == 1),
                )
            go = wpool.tile([G, D_OUT], f32, tag="go")
            nc.vector.tensor_tensor(
                out=go[:], in0=ph2[:], in1=b2_s[:G, :], op=Alu.add
            )
            nc.scalar.activation(go[:], go[:], AFT.Lrelu, alpha=0.01)
            nc.sync.dma_start(out_d[:], go[:])

    nc.compile()
    return nc


def _install_ntff_shim():
    """Provide antenv.axon_hooks (missing in this image) so that
    run_bass_kernel_spmd(trace=True) can capture an NTFF profile via the
    injected libaxon_pjrt.so. Only used when TRACE=True."""
    import types
    import ctypes
    import contextlib

    try:
        from antenv.axon_hooks import get_axon_ntff_profile_hook  # noqa: F401
        return
    except ImportError:
        pass
    so_path = "/opt/axon/libaxon_pjrt.so"
    try:
        lib = ctypes.CDLL(so_path)
    except OSError:
        return
    if not hasattr(lib, "axon_start_nrt_profile"):
        return
    lib.axon_start_nrt_profile.argtypes = [
        ctypes.POINTER(ctypes.c_int64), ctypes.c_size_t,
    ]
    lib.axon_start_nrt_profile.restype = ctypes.c_int64
    lib.axon_stop_nrt_profile.argtypes = [ctypes.c_char_p]
    lib.axon_stop_nrt_profile.restype = ctypes.c_int64

    @contextlib.contextmanager
    def _hook(output_dir, device_ids):
        import jax
        jax.devices()
        if device_ids:
            ids = (ctypes.c_int64 * len(device_ids))(*device_ids)
            rc = lib.axon_start_nrt_profile(ids, len(device_ids))
        else:
            rc = lib.axon_start_nrt_profile(None, 0)
        if rc != 0:
            raise RuntimeError(f"axon_start_nrt_profile rc={rc}")
        try:
            yield
        finally:
            n = lib.axon_stop_nrt_profile(str(output_dir).encode())
            print(f"profile: {n} file(s) written to {output_dir}",
                  file=sys.stderr)

    mod = types.ModuleType("antenv.axon_hooks")
    mod.get_axon_ntff_profile_hook = lambda: _hook
    mod.set_axon_ntff_profile_hook = lambda h: None
    sys.modules["antenv.axon_hooks"] = mod


def kernel(**inputs):
    global LAST_RESULTS
    if TRACE:
        _install_ntff_shim()
    ins = {k: np.asarray(v) for k, v in inputs.items()}
    cfg, common, per_core = _host_prep(
        ins["x"].astype(np.float32), ins["edge_index"], ins["batch"],
        ins["W_emb"], ins["b_emb"], ins["W_conv"], ins["b_conv"],
        ins["W1"], ins["b1"], ins["W2"], ins["b2"],
    )
    nc = _build(cfg)

    in_maps = []
    for c in range(C):
        m = dict(
            xT=per_core[c]["xT"],
            W_emb=common["W_emb"], W_conv=common["W_conv"],
            W1=common["W1"], W2=common["W2"],
            b_emb_b=common["b_emb_b"], b_conv_b=common["b_conv_b"],
            b1_b=common["b1_b"], b2_b=common["b2_b"],
            iota=common["iota"], ident=common["ident"],
            identb=common["identb"],
            cntinv=common["cntinv"],
            dinv_t=per_core[c]["dinv_t"], idx=per_core[c]["idx"],
            dstloc=per_core[c]["dstloc"], gid=per_core[c]["gid"],
            mult_t=per_core[c]["mult_t"],
        )
        in_maps.append(m)

    res = run_bass_kernel_spmd(
        nc, in_maps, core_ids=list(range(C)), trace=TRACE
    )
    LAST_RESULTS = res
    return np.asarray(res.results[0]["out"], dtype=np.float32)

